# revision 1
# baseline (speedup 1.0000x reference)
"""Trainium2 Bass kernel for nn_BlockWithAttention (dense CNN block + attention).

Sharding: data-parallel over batch (B=16 -> 2 samples/core x 8 cores).
BatchNorm batch statistics are synced with two tiny HBM AllGathers
([128,4] per core) + local DVE reduction. All matmuls run in float32r
(full PE rate, ~1e-4 relative precision); accumulation is fp32 in PSUM.

Self-contained: hardcodes shapes; only needs concourse (on PYTHONPATH in
this container) + numpy.
"""
import numpy as np

import concourse.bass as bass
import concourse.mybir as mybir
from concourse.bass_utils import run_bass_kernel_spmd
from concourse.tile import TileContext
from concourse.tile_rust import add_dep_helper

# ---- problem constants ----
B, C, H, W, T, CQ = 16, 256, 32, 32, 256, 32
NCORES = 8
BL = B // NCORES            # samples per core
KT = C // 128               # 128-channel tiles
HP, WP = H + 2, W + 2       # padded image
NPAD = HP * WP              # 1156
NPIX = B * H * W            # BN stat count (full batch)
N = H * W                   # 1024 spatial positions
RH = 16                     # rows per 512-px half
EPS = 1e-5

F32 = mybir.dt.float32
F32R = mybir.dt.float32r
AX = mybir.AxisListType
ALU = mybir.AluOpType
AF = mybir.ActivationFunctionType

_wsplit_counter = [0]


def _split_packed_waits(nc, max_waits: int = 1):
    """The walrus build here rejects >1-2 packed sync-waits per instruction
    ("Too many sync wait commands"). Move excess waits onto standalone
    single-wait EventSemaphore carriers inserted before the instruction
    (same engine -> program order preserves gating)."""
    for f in nc.m.functions:
        for bb in f.blocks:
            il = bb.instructions
            i = 0
            while i < len(il):
                inst = il[i]
                si = inst.sync_info
                if si is not None and len(si.on_wait) > max_waits:
                    waits = list(si.on_wait)
                    movable = [w for w in waits if w.wait_reg is None]
                    fixed = [w for w in waits if w.wait_reg is not None]
                    keep_n = max(0, max_waits - len(fixed))
                    kept = fixed + movable[:keep_n]
                    move = movable[keep_n:]
                    if not move:
                        i += 1
                        continue
                    si.on_wait = kept
                    for w in move:
                        _wsplit_counter[0] += 1
                        ev = mybir.InstEventSemaphore(
                            name=f"I-wsplit-{_wsplit_counter[0]}",
                            opcode="EventSemaphore",
                            engine=inst.engine,
                            sync_info=mybir.SyncInfo(on_wait=[w], on_update=[]),
                        )
                        il.insert(i, ev)
                        i += 1
                i += 1


def _pad3(tile):
    """[128, NPAD] pad tile viewed as [128, HP, WP]."""
    return tile[:, :].rearrange("p (r c) -> p r c", c=WP)


def _interior(tile, r0=0, nr=H):
    """interior rows r0..r0+nr of the HxW image inside a pad tile."""
    return _pad3(tile)[:, 1 + r0:1 + r0 + nr, 1:1 + W]


def _tap(tile, dy, dx, r0, nr):
    """conv tap read: out rows [r0, r0+nr) <- pad rows [r0+dy, ...)."""
    return _pad3(tile)[:, r0 + dy:r0 + dy + nr, dx:dx + W]


U32 = mybir.dt.uint32
ONE_F32_BITS = 0x3F800000


def _memset_border(nc, tile):
    # gpsimd memset rejects float32r in this walrus build; write via a
    # uint32 bitcast (identical bits)
    v = _pad3(tile)
    nc.gpsimd.memset(v[:, 0:1, :].bitcast(U32), 0)
    nc.gpsimd.memset(v[:, HP - 1:HP, :].bitcast(U32), 0)
    nc.gpsimd.memset(v[:, 1:HP - 1, 0:1].bitcast(U32), 0)
    nc.gpsimd.memset(v[:, 1:HP - 1, WP - 1:WP].bitcast(U32), 0)


def build(split: bool = True, dt_conv=F32R, dt_attn=F32R,
          exp_shift: float = 0.0, exact_sq: bool = False, nr_rsqrt: bool = False,
          no_cc: bool = False):
    nc = bass.Bass(num_devices=NCORES)

    # ---- DRAM I/O ----
    xp_d = nc.dram_tensor("xp", [BL, KT, 128, NPAD], dt_conv, kind="ExternalInput")
    cw_d = nc.dram_tensor("cw", [3, 128, 9 * KT * KT * 128], dt_conv, kind="ExternalInput")
    w1t_d = nc.dram_tensor("w1t", [KT, 128, T], F32R, kind="ExternalInput")
    w2t_d = nc.dram_tensor("w2t", [KT, 128, C], F32R, kind="ExternalInput")
    # packed per-channel constants: cols 0-5 conv biases (ci*2+k),
    # 6-9 bn gammas (i*2+k), 10-13 bn betas, 14-15 b_t1, 16-17 b_t2,
    # 18-21 t^T per-core slices (k*BL+s)
    consts_d = nc.dram_tensor("consts", [128, 22], F32R, kind="ExternalInput")
    wqt_d = nc.dram_tensor("wqt", [KT, 128, CQ], dt_attn, kind="ExternalInput")
    wkt_d = nc.dram_tensor("wkt", [KT, 128, CQ], dt_attn, kind="ExternalInput")
    wvt_d = nc.dram_tensor("wvt", [KT, 128, C], dt_attn, kind="ExternalInput")
    bq_d = nc.dram_tensor("bq", [CQ, 1], F32R, kind="ExternalInput")
    bk_d = nc.dram_tensor("bk", [CQ, 1], F32R, kind="ExternalInput")
    bv_d = nc.dram_tensor("bv", [1, C], dt_attn, kind="ExternalInput")
    gam_d = nc.dram_tensor("gam", [1, 1], F32, kind="ExternalInput")
    out_d = nc.dram_tensor("out", [BL, KT, 128, N], F32R, kind="ExternalOutput")

    # collective bounce buffers (HBM-HBM); AllGather (15us modeled) beats
    # AllReduce (28us modeled) for this tiny payload - reduce locally on DVE
    cc_in = [nc.dram_tensor(f"cc{i}_in", [128, 4], F32) for i in range(2)]
    cc_out = [nc.dram_tensor(f"cc{i}_out", [NCORES, 128, 4], F32, addr_space="Shared")
              for i in range(2)]

    with TileContext(nc) as tc:
        with (
            tc.tile_pool(name="pconst", bufs=1) as pc,
            tc.tile_pool(name="pcw", bufs=3) as pcw,
            tc.tile_pool(name="ppad", bufs=8) as ppad,
            tc.tile_pool(name="py", bufs=4) as py,
            tc.tile_pool(name="psq", bufs=2) as psq,
            tc.tile_pool(name="pattn", bufs=1) as pat,
            tc.tile_pool(name="pstats", bufs=1) as pst,
            tc.tile_pool(name="ppsum", bufs=8, space="PSUM") as pps,
        ):
            def psum(nm):
                return pps.tile([128, 512], F32, tag="ps", name=nm)

            # ---- big DMAs first: conv1 weights + padded inputs, so the PE
            # can start as early as possible (each SP dma_start costs ~565ns
            # of sequencer time; small constants go via gpsimd SWDGE) ----
            cw_sb = []
            for ci in range(3):
                t = pcw.tile([128, 9 * KT * KT * 128], dt_conv, tag="cw", name=f"cw{ci}")
                if ci == 0:
                    nc.sync.dma_start(out=t[:, :], in_=cw_d[ci, :, :])
                cw_sb.append(t)
            x_pad = [[ppad.tile([128, NPAD], dt_conv, tag="pad", name=f"xp{s}{k}")
                      for k in range(KT)] for s in range(BL)]
            for s in range(BL):
                for k in range(KT):
                    nc.sync.dma_start(out=x_pad[s][k][:, :], in_=xp_d[s, k, :, :])

            # ---- persistent small tiles ----
            w1t_sb = [pc.tile([128, T], F32R, name=f"w1t{k}") for k in range(KT)]
            w2t_sb = [pc.tile([128, C], F32R, name=f"w2t{k}") for k in range(KT)]
            consts_sb = pc.tile([128, 22], F32R, name="consts_sb")

            def ccol(j, n=1):
                return consts_sb[:, j:j + n]

            cb_sb = [[ccol(ci * KT + k) for k in range(KT)] for ci in range(3)]
            bng_sb = [[ccol(6 + i * KT + k) for k in range(KT)] for i in range(2)]
            bnb_sb = [[ccol(10 + i * KT + k) for k in range(KT)] for i in range(2)]
            bt1_sb = [ccol(14 + k) for k in range(KT)]
            bt2_sb = [ccol(16 + k) for k in range(KT)]
            tt_sb = [ccol(18 + k * BL, BL) for k in range(KT)]
            wqt_sb = [pc.tile([128, CQ], dt_attn, name=f"wqt{k}") for k in range(KT)]
            wkt_sb = [pc.tile([128, CQ], dt_attn, name=f"wkt{k}") for k in range(KT)]
            wvt_sb = [pc.tile([128, C], dt_attn, name=f"wvt{k}") for k in range(KT)]
            bq_sb = pc.tile([CQ, 1], F32R, name="bq_sb")
            bk_sb = pc.tile([CQ, 1], F32R, name="bk_sb")
            bv_sb = pc.tile([1, C], dt_attn, name="bv_sb")
            gam_sb = pc.tile([1, 1], F32, name="gam_sb")
            ones_col = pc.tile([128, 1], dt_attn, name="ones_col")
            ones_row = pc.tile([1, 128], dt_attn, name="ones_row")
            expb_sb = None
            if exp_shift != 0.0:
                expb_sb = pc.tile([128, 1], F32, name="expb_sb")
                bits = int.from_bytes(np.float32(exp_shift).tobytes(), "little")
                nc.gpsimd.memset(expb_sb[:, :].bitcast(U32), bits)

            nc.gpsimd.dma_start(out=consts_sb[:, :], in_=consts_d[:, :])
            for k in range(KT):
                nc.sync.dma_start(out=w1t_sb[k][:, :], in_=w1t_d[k, :, :])
                nc.sync.dma_start(out=w2t_sb[k][:, :], in_=w2t_d[k, :, :])
                nc.sync.dma_start(out=wqt_sb[k][:, :], in_=wqt_d[k, :, :])
                nc.sync.dma_start(out=wkt_sb[k][:, :], in_=wkt_d[k, :, :])
                nc.sync.dma_start(out=wvt_sb[k][:, :], in_=wvt_d[k, :, :])
            nc.gpsimd.dma_start(out=bq_sb[:, :], in_=bq_d[:, :])
            nc.gpsimd.dma_start(out=bk_sb[:, :], in_=bk_d[:, :])
            nc.gpsimd.dma_start(out=bv_sb[:, :], in_=bv_d[:, :])
            nc.gpsimd.dma_start(out=gam_sb[:, :], in_=gam_d[:, :])
            nc.gpsimd.memset(ones_col[:, :].bitcast(U32), ONE_F32_BITS)
            nc.gpsimd.memset(ones_row[:, :].bitcast(U32), ONE_F32_BITS)
            # conv2/conv3 weights stream in behind conv1's
            for ci in range(1, 3):
                nc.sync.dma_start(out=cw_sb[ci][:, :], in_=cw_d[ci, :, :])

            # stats: cols [0:8]=sum(ko,s,half), [8:16]=sumsq(ko,s,half)
            stats = [pst.tile([128, 16], F32, name=f"stats{i}") for i in range(2)]
            ccp = [pst.tile([128, 4], F32, name=f"ccp{i}") for i in range(2)]
            glob = [pst.tile([128, 4], F32, name=f"glob{i}") for i in range(2)]
            gall = [pst.tile([128, 4 * NCORES], F32, name=f"gall{i}") for i in range(2)]
            for i in range(2):
                nc.gpsimd.memset(stats[i][:, :], 0.0)

            # pad buffers
            h1_pad = [[ppad.tile([128, NPAD], dt_conv, tag="pad", name=f"h1p{s}{k}")
                       for k in range(KT)] for s in range(BL)]
            for s in range(BL):
                for k in range(KT):
                    _memset_border(nc, h1_pad[s][k])

            # ---- conv + stats helper ----
            def conv(ci, src_pads, s, ko, half, epilogue):
                ps = psum(f"cps{ci}_{s}{ko}{half}")
                ps3 = ps[:, :].rearrange("p (r c) -> p r c", c=W)
                r0 = half * RH
                idx = 0
                for tap in range(9):
                    dy, dx = divmod(tap, 3)
                    for ki in range(KT):
                        j = (tap * KT + ki) * KT + ko
                        nc.tensor.matmul(
                            ps3,
                            cw_sb[ci][:, j * 128:(j + 1) * 128],
                            _tap(src_pads[s][ki], dy, dx, r0, RH),
                            start=(idx == 0), stop=(idx == 17),
                        )
                        idx += 1
                epilogue(ps, ps3, r0)

            # ---- conv1 -> relu -> (stats) -> h1_pad interior (raw) ----
            for s in range(BL):
                for ko in range(KT):
                    for half in range(2):
                        def epi1(ps, ps3, r0, s=s, ko=ko, half=half):
                            nc.scalar.activation(
                                _interior(h1_pad[s][ko], r0, RH), ps3, AF.Relu,
                                bias=cb_sb[0][ko][:, :],
                                accum_out=stats[0][:, ko * 4 + s * 2 + half:
                                                   ko * 4 + s * 2 + half + 1],
                            )
                            sq = psq.tile([128, 512], F32, tag="sq",
                                          name=f"sq1_{s}{ko}{half}")
                            nc.scalar.activation(
                                sq[:, :].rearrange("p (r c) -> p r c", c=W),
                                _interior(h1_pad[s][ko], r0, RH), AF.Square,
                                accum_out=stats[0][:, 8 + ko * 4 + s * 2 + half:
                                                   9 + ko * 4 + s * 2 + half],
                            )
                        conv(0, x_pad, s, ko, half, epi1)


            # ---- time MLP (independent; scheduler fills gaps) ----
            te1_sb = [pst.tile([128, BL], F32R, name=f"te1_{m}") for m in range(KT)]
            te_sb = [pst.tile([128, BL], F32R, name=f"te_{m}") for m in range(KT)]
            for mo in range(KT):
                ps = psum(f"mlp1_{mo}")
                for ki in range(KT):
                    nc.tensor.matmul(ps[:, 0:BL],
                                     w1t_sb[ki][:, mo * 128:(mo + 1) * 128],
                                     tt_sb[ki][:, :],
                                     start=(ki == 0), stop=(ki == KT - 1))
                nc.scalar.activation(te1_sb[mo][:, :], ps[:, 0:BL], AF.Relu,
                                     bias=bt1_sb[mo][:, :])
            for mo in range(KT):
                ps = psum(f"mlp2_{mo}")
                for ki in range(KT):
                    nc.tensor.matmul(ps[:, 0:BL],
                                     w2t_sb[ki][:, mo * 128:(mo + 1) * 128],
                                     te1_sb[ki][:, :],
                                     start=(ki == 0), stop=(ki == KT - 1))
                nc.scalar.activation(te_sb[mo][:, :], ps[:, 0:BL], AF.Relu,
                                     bias=bt2_sb[mo][:, :])

            # ---- BN stat sync + normalization constants ----
            def bn_sync(i):
                for ko in range(KT):
                    nc.vector.reduce_sum(ccp[i][:, ko * 2:ko * 2 + 1],
                                         stats[i][:, ko * 4:ko * 4 + 4], axis=AX.X)
                    nc.vector.reduce_sum(ccp[i][:, ko * 2 + 1:ko * 2 + 2],
                                         stats[i][:, 8 + ko * 4:12 + ko * 4], axis=AX.X)
                if no_cc:  # timing-ablation only: skip the sync (wrong stats scale)
                    nc.vector.tensor_scalar_mul(glob[i][:, :], ccp[i][:, :], 8.0)
                    return
                d1 = nc.gpsimd.dma_start(out=cc_in[i][:, :], in_=ccp[i][:, :])
                cc = nc.gpsimd.collective_compute(
                    "AllGather", ALU.bypass,
                    replica_groups=[list(range(NCORES))],
                    ins=[cc_in[i][:].opt()], outs=[cc_out[i][:].opt()],
                )
                add_dep_helper(cc.ins, d1.ins, reason="cc waits on stats dma")
                d2 = nc.gpsimd.dma_start(
                    out=gall[i][:, :],
                    in_=cc_out[i][:, :, :].rearrange("c p k -> p c k"))
                add_dep_helper(d2.ins, cc.ins, reason="readback waits on cc")
                # sum the 8 per-core partials: view [p, k, c], reduce over c
                nc.vector.reduce_sum(
                    glob[i][:, :],
                    gall[i][:, :].rearrange("p (c k) -> p k c", k=4), axis=AX.X)

            def bn_consts(i):
                """returns per-ko (scale, shift) tiles"""
                out = []
                for ko in range(KT):
                    mean = pst.tile([128, 1], F32, name=f"mean{i}{ko}")
                    ex2 = pst.tile([128, 1], F32, name=f"ex2{i}{ko}")
                    var = pst.tile([128, 1], F32, name=f"var{i}{ko}")
                    rv = pst.tile([128, 1], F32, name=f"rv{i}{ko}")
                    scl = pst.tile([128, 1], F32, name=f"scl{i}{ko}")
                    shf = pst.tile([128, 1], F32, name=f"shf{i}{ko}")
                    nc.vector.tensor_scalar_mul(mean[:, :], glob[i][:, ko * 2:ko * 2 + 1],
                                                1.0 / NPIX)
                    nc.vector.tensor_scalar_mul(ex2[:, :], glob[i][:, ko * 2 + 1:ko * 2 + 2],
                                                1.0 / NPIX)
                    nc.vector.tensor_tensor(var[:, :], mean[:, :], mean[:, :], ALU.mult)
                    nc.vector.tensor_tensor(var[:, :], ex2[:, :], var[:, :], ALU.subtract)
                    nc.vector.tensor_scalar(out=var[:, :], in0=var[:, :], scalar1=EPS,
                                            scalar2=None, op0=ALU.add)
                    nc.vector.reciprocal(rv[:, :], var[:, :])
                    nc.scalar.activation(rv[:, :], rv[:, :], AF.Sqrt)  # rsqrt(var+eps)
                    if nr_rsqrt:
                        # Newton step for y ~ rsqrt(v): y' = 0.5*y*(3 - v*y^2)
                        t1 = pst.tile([128, 1], F32, name=f"nr1_{i}{ko}")
                        nc.vector.tensor_tensor(t1[:, :], rv[:, :], rv[:, :], ALU.mult)
                        nc.vector.tensor_tensor(t1[:, :], var[:, :], t1[:, :], ALU.mult)
                        nc.vector.tensor_scalar(out=t1[:, :], in0=t1[:, :], scalar1=-1.0,
                                                scalar2=3.0, op0=ALU.mult, op1=ALU.add)
                        nc.vector.tensor_tensor(t1[:, :], rv[:, :], t1[:, :], ALU.mult)
                        nc.vector.tensor_scalar_mul(rv[:, :], t1[:, :], 0.5)
                    nc.vector.tensor_tensor(scl[:, :], rv[:, :], bng_sb[i][ko][:, :],
                                            ALU.mult)
                    nc.vector.tensor_tensor(shf[:, :], mean[:, :], scl[:, :], ALU.mult)
                    nc.vector.tensor_tensor(shf[:, :], bnb_sb[i][ko][:, :], shf[:, :],
                                            ALU.subtract)
                    out.append((scl, shf))
                return out

            bn_sync(0)
            bn1 = bn_consts(0)

            # normalize h1 in place (+ te per sample)
            for s in range(BL):
                for ko in range(KT):
                    bsk = pst.tile([128, 1], F32, name=f"b1s{s}{ko}")
                    nc.vector.tensor_tensor(bsk[:, :], bn1[ko][1][:, :],
                                            te_sb[ko][:, s:s + 1], ALU.add)
                    nc.scalar.activation(_interior(h1_pad[s][ko]),
                                         _interior(h1_pad[s][ko]), AF.Identity,
                                         bias=bsk[:, :], scale=bn1[ko][0][:, :])

            # ---- conv2 -> relu -> stats -> h2_pad (raw) ----
            h2_pad = [[ppad.tile([128, NPAD], dt_conv, tag="pad", name=f"h2p{s}{k}")
                       for k in range(KT)] for s in range(BL)]
            for s in range(BL):
                for k in range(KT):
                    _memset_border(nc, h2_pad[s][k])
            for s in range(BL):
                for ko in range(KT):
                    for half in range(2):
                        def epi2(ps, ps3, r0, s=s, ko=ko, half=half):
                            nc.scalar.activation(
                                _interior(h2_pad[s][ko], r0, RH), ps3, AF.Relu,
                                bias=cb_sb[1][ko][:, :],
                                accum_out=stats[1][:, ko * 4 + s * 2 + half:
                                                   ko * 4 + s * 2 + half + 1],
                            )
                            sq = psq.tile([128, 512], F32, tag="sq",
                                          name=f"sq2_{s}{ko}{half}")
                            nc.scalar.activation(
                                sq[:, :].rearrange("p (r c) -> p r c", c=W),
                                _interior(h2_pad[s][ko], r0, RH), AF.Square,
                                accum_out=stats[1][:, 8 + ko * 4 + s * 2 + half:
                                                   9 + ko * 4 + s * 2 + half],
                            )
                        conv(1, h1_pad, s, ko, half, epi2)


            bn_sync(1)
            bn2 = bn_consts(1)
            for s in range(BL):
                for ko in range(KT):
                    nc.scalar.activation(_interior(h2_pad[s][ko]),
                                         _interior(h2_pad[s][ko]), AF.Identity,
                                         bias=bn2[ko][1][:, :], scale=bn2[ko][0][:, :])

            # ---- conv3 (transform; bias, no relu) -> y tiles ----
            y_sb = [[py.tile([128, N], dt_attn, tag="y", name=f"y{s}{k}")
                     for k in range(KT)] for s in range(BL)]
            for s in range(BL):
                for ko in range(KT):
                    for half in range(2):
                        def epi3(ps, ps3, r0, s=s, ko=ko, half=half):
                            nc.scalar.activation(
                                y_sb[s][ko][:, half * 512:(half + 1) * 512],
                                ps[:, :], AF.Identity, bias=cb_sb[2][ko][:, :])
                        conv(2, h2_pad, s, ko, half, epi3)

            # ---- attention (per sample) ----
            for s in range(BL):
                # V^T tiles: [n-tile 128, C]
                vt = []
                for nt in range(8):
                    ps = psum(f"vps{s}{nt}")
                    pv = ps[:, 0:C]
                    for c2 in range(KT):
                        nc.tensor.matmul(pv, y_sb[s][c2][:, nt * 128:(nt + 1) * 128],
                                         wvt_sb[c2][:, :], start=(c2 == 0), stop=False)
                    nc.tensor.matmul(pv, ones_row[:, :], bv_sb[:, :],
                                     start=False, stop=True)
                    v = pat.tile([128, C], dt_attn, tag="vt", bufs=9, name=f"vt{s}{nt}")
                    nc.vector.tensor_copy(v[:, :], pv)
                    vt.append(v)

                # Q, K: [CQ, N]
                q_sb = pat.tile([CQ, N], dt_attn, tag="q", bufs=2, name=f"q{s}")
                k_sb = pat.tile([CQ, N], dt_attn, tag="k", bufs=2, name=f"k{s}")
                for nh in range(2):
                    psq_ = psum(f"qps{s}{nh}")
                    for c2 in range(KT):
                        nc.tensor.matmul(psq_[0:CQ, :], wqt_sb[c2][:, :],
                                         y_sb[s][c2][:, nh * 512:(nh + 1) * 512],
                                         start=(c2 == 0), stop=(c2 == KT - 1))
                    nc.scalar.activation(q_sb[:, nh * 512:(nh + 1) * 512],
                                         psq_[0:CQ, :], AF.Identity, bias=bq_sb[:, :])
                    psk_ = psum(f"kps{s}{nh}")
                    for c2 in range(KT):
                        nc.tensor.matmul(psk_[0:CQ, :], wkt_sb[c2][:, :],
                                         y_sb[s][c2][:, nh * 512:(nh + 1) * 512],
                                         start=(c2 == 0), stop=(c2 == KT - 1))
                    nc.scalar.activation(k_sb[:, nh * 512:(nh + 1) * 512],
                                         psk_[0:CQ, :], AF.Identity, bias=bk_sb[:, :])

                res_t = [pat.tile([128, N], F32R, tag="res", bufs=4,
                                  name=f"res{s}{c2}") for c2 in range(KT)]
                for nh in range(2):
                    # S^T tiles -> P = exp(S^T)  (no max-shift: |logits| << 80)
                    ptiles = []
                    for mt in range(8):
                        ps = psum(f"sps{s}{nh}{mt}")
                        nc.tensor.matmul(ps[:, :], k_sb[:, mt * 128:(mt + 1) * 128],
                                         q_sb[:, nh * 512:(nh + 1) * 512],
                                         start=True, stop=True)
                        p = pat.tile([128, 512], dt_attn, tag="P", bufs=16,
                                     name=f"P{s}{nh}{mt}")
                        nc.scalar.activation(
                            p[:, :], ps[:, :], AF.Exp,
                            bias=expb_sb[:, :] if exp_shift != 0.0 else 0.0)
                        ptiles.append(p)
                    # denom[n] = sum_m P: DVE add-tree across m-tiles (keeps
                    # PE free), then one ones-matmul for the partition reduce
                    pacc = pat.tile([128, 512], dt_attn, tag="pacc", bufs=2,
                                    name=f"pacc{s}{nh}")
                    nc.vector.tensor_tensor(pacc[:, :], ptiles[0][:, :],
                                            ptiles[1][:, :], ALU.add)
                    for mt in range(2, 8):
                        nc.vector.tensor_tensor(pacc[:, :], pacc[:, :],
                                                ptiles[mt][:, :], ALU.add)
                    pd = psum(f"dps{s}{nh}")
                    nc.tensor.matmul(pd[0:1, :], ones_col[:, :], pacc[:, :],
                                     start=True, stop=True)
                    rcp = pat.tile([1, 512], dt_attn, tag="rcp", bufs=2, name=f"rcp{s}{nh}")
                    with nc.allow_low_precision(reason="f32r==f32 bit layout"):
                        nc.vector.reciprocal(rcp[:, :], pd[0:1, :])
                    nc.vector.tensor_scalar(out=rcp[:, :], in0=rcp[:, :],
                                            scalar1=gam_sb[0:1, 0:1], scalar2=None,
                                            op0=ALU.mult)
                    # broadcast gamma/denom down partitions
                    pb = psum(f"bps{s}{nh}")
                    nc.tensor.matmul(pb[:, :], ones_row[:, :], rcp[:, :],
                                     start=True, stop=True)
                    rb = pat.tile([128, 512], F32, tag="rb", bufs=2, name=f"rb{s}{nh}")
                    nc.vector.tensor_copy(rb[:, :], pb[:, :])
                    # out = (V @ P) * rb + y
                    for c2 in range(KT):
                        pr = psum(f"rps{s}{nh}{c2}")
                        for mt in range(8):
                            nc.tensor.matmul(pr[:, :],
                                             vt[mt][:, c2 * 128:(c2 + 1) * 128],
                                             ptiles[mt][:, :],
                                             start=(mt == 0), stop=(mt == 7))
                        rs = res_t[c2][:, nh * 512:(nh + 1) * 512]
                        nc.vector.tensor_tensor(rs, pr[:, :], rb[:, :], ALU.mult)
                        nc.vector.tensor_tensor(rs, rs,
                                                y_sb[s][c2][:, nh * 512:(nh + 1) * 512],
                                                ALU.add)
                for c2 in range(KT):
                    nc.sync.dma_start(out=out_d[s, c2, :, :], in_=res_t[c2][:, :])

    if split:
        _split_packed_waits(nc)
    return nc


def _prep_inputs(inputs):
    """host-side reshape/transpose; returns (shared_map, per_core_maps)"""
    f32 = np.float32
    x = np.asarray(inputs["x"], f32)
    t = np.asarray(inputs["t"], f32)

    def conv_w(w):
        w6 = np.asarray(w, f32).reshape(KT, 128, KT, 128, 3, 3)  # ko,o,ki,i,dy,dx
        arr = w6.transpose(3, 4, 5, 2, 0, 1)  # i,dy,dx,ki,ko,o
        return np.ascontiguousarray(arr.reshape(128, 9 * KT * KT * 128))

    cw = np.stack([conv_w(inputs["w_c1"]), conv_w(inputs["w_c2"]),
                   conv_w(inputs["w_tr"])])
    w1t = np.ascontiguousarray(np.asarray(inputs["w_t1"], f32).T.reshape(KT, 128, T))
    w2t = np.ascontiguousarray(np.asarray(inputs["w_t2"], f32).T.reshape(KT, 128, C))
    # packed per-channel constants (see consts_d layout in build())
    consts = np.zeros((128, 22), f32)
    for ci, k2 in enumerate(("b_c1", "b_c2", "b_tr")):
        consts[:, ci * KT:(ci + 1) * KT] = np.asarray(inputs[k2], f32).reshape(KT, 128).T
    for i, (gk, bk2) in enumerate((("bn1_g", "bn1_b"), ("bn2_g", "bn2_b"))):
        consts[:, 6 + i * KT:6 + (i + 1) * KT] = np.asarray(inputs[gk], f32).reshape(KT, 128).T
        consts[:, 10 + i * KT:10 + (i + 1) * KT] = np.asarray(inputs[bk2], f32).reshape(KT, 128).T
    consts[:, 14:16] = np.asarray(inputs["b_t1"], f32).reshape(KT, 128).T
    consts[:, 16:18] = np.asarray(inputs["b_t2"], f32).reshape(KT, 128).T
    wqt = np.ascontiguousarray(np.asarray(inputs["wq"], f32).T.reshape(KT, 128, CQ))
    wkt = np.ascontiguousarray(np.asarray(inputs["wk"], f32).T.reshape(KT, 128, CQ))
    wvt = np.ascontiguousarray(np.asarray(inputs["wv"], f32).T.reshape(KT, 128, C))
    bq = np.asarray(inputs["bq"], f32).reshape(CQ, 1)
    bk = np.asarray(inputs["bk"], f32).reshape(CQ, 1)
    bv = np.asarray(inputs["bv"], f32).reshape(1, C)
    gam = np.asarray(inputs["gamma"], f32).reshape(1, 1)

    xp = np.zeros((B, KT, 128, HP, WP), f32)
    xp[:, :, :, 1:1 + H, 1:1 + W] = x.reshape(B, KT, 128, H, W)
    xp = xp.reshape(B, KT, 128, NPAD)
    ttr = np.ascontiguousarray(t.T.reshape(KT, 128, B))

    shared = dict(cw=cw, w1t=w1t, w2t=w2t,
                  wqt=wqt, wkt=wkt, wvt=wvt, bq=bq, bk=bk, bv=bv, gam=gam)
    per_core = []
    for c in range(NCORES):
        m = dict(shared)
        m["xp"] = np.ascontiguousarray(xp[c * BL:(c + 1) * BL])
        cc_consts = consts.copy()
        for k in range(KT):
            cc_consts[:, 18 + k * BL:18 + (k + 1) * BL] = \
                ttr[k, :, c * BL:(c + 1) * BL]
        m["consts"] = cc_consts
        per_core.append(m)
    return per_core


def _unshard(results):
    out = np.empty((B, C, H, W), np.float32)
    for c in range(NCORES):
        o = results[c]["out"].reshape(BL, KT, 128, H, W)
        for s in range(BL):
            out[c * BL + s] = o[s].reshape(C, H, W)
    return out


_cache = {}


DT_CONV = F32R
DT_ATTN = F32R


def kernel(**inputs) -> np.ndarray:
    key = ("nc", str(DT_CONV), str(DT_ATTN))
    if key not in _cache:
        _cache[key] = build(dt_conv=DT_CONV, dt_attn=DT_ATTN, nr_rsqrt=True)
    nc = _cache[key]
    per_core = _prep_inputs(inputs)
    try:
        res = run_bass_kernel_spmd(nc, per_core, core_ids=list(range(NCORES)))
    except Exception:
        # transient NRT_EXEC_UNIT_UNRECOVERABLE errors recover on re-execute
        res = run_bass_kernel_spmd(nc, per_core, core_ids=list(range(NCORES)))
    return _unshard(res.results)



# revision 2
# speedup vs baseline: 1.1904x; 1.1904x over previous
"""Trainium2 Bass kernel for nn_BlockWithAttention (dense CNN block + attention).

Sharding: data-parallel over batch (B=16 -> 2 samples/core x 8 cores).

Key scheduling ideas vs the naive version:
- BN batch-stat sync is chunked per 128-channel group (2 AllGathers per BN,
  each a flat ~15us on the modeled collective device) and pipelined against
  conv matmuls: chunk-ko0's collective flies while PE does conv-ko1, and
  BN1-ko1's collective flies while PE runs conv2's ki=0 partial sums.
- conv2/conv3 are split into ki phases (psums stay open across the phase
  boundary) so PE work exists before the second BN chunk lands.
- attention emits the V@P output matmuls *before* the softmax-denominator
  matmuls so PE never stalls on the DVE add-tree.
- DMA order: x tiles first, conv1 weights in ko-major halves, so PE starts
  ~7us in instead of ~11.5us.

All matmuls run in float32r (full PE rate at free-size>=256, fp32-identical
bit layout on SBUF); accumulation is fp32 in PSUM.
"""
import numpy as np

import concourse.bass as bass
import concourse.mybir as mybir
from concourse.bass_utils import run_bass_kernel_spmd
from concourse.tile import TileContext
from concourse.tile_rust import add_dep_helper

# ---- problem constants ----
B, C, H, W, T, CQ = 16, 256, 32, 32, 256, 32
NCORES = 8
BL = B // NCORES            # samples per core
KT = C // 128               # 128-channel tiles
HP, WP = H + 2, W + 2       # padded image
NPAD = HP * WP              # 1156
NPIX = B * H * W            # BN stat count (full batch)
N = H * W                   # 1024 spatial positions
RH = 16                     # rows per 512-px half
EPS = 1e-5
KCOLS = 9 * KT * 128        # weight cols per ko group (ko-major layout)

F32 = mybir.dt.float32
F32R = mybir.dt.float32r
AX = mybir.AxisListType
ALU = mybir.AluOpType
AF = mybir.ActivationFunctionType

U32 = mybir.dt.uint32
ONE_F32_BITS = 0x3F800000

_wsplit_counter = [0]


def _split_packed_waits(nc, max_waits: int = 1):
    """The walrus build here rejects >1-2 packed sync-waits per instruction
    ("Too many sync wait commands"). Move excess waits onto standalone
    single-wait EventSemaphore carriers inserted before the instruction
    (same engine -> program order preserves gating)."""
    for f in nc.m.functions:
        for bb in f.blocks:
            il = bb.instructions
            i = 0
            while i < len(il):
                inst = il[i]
                si = inst.sync_info
                if si is not None and len(si.on_wait) > max_waits:
                    waits = list(si.on_wait)
                    movable = [w for w in waits if w.wait_reg is None]
                    fixed = [w for w in waits if w.wait_reg is not None]
                    keep_n = max(0, max_waits - len(fixed))
                    kept = fixed + movable[:keep_n]
                    move = movable[keep_n:]
                    if not move:
                        i += 1
                        continue
                    si.on_wait = kept
                    for w in move:
                        _wsplit_counter[0] += 1
                        ev = mybir.InstEventSemaphore(
                            name=f"I-wsplit-{_wsplit_counter[0]}",
                            opcode="EventSemaphore",
                            engine=inst.engine,
                            sync_info=mybir.SyncInfo(on_wait=[w], on_update=[]),
                        )
                        il.insert(i, ev)
                        i += 1
                i += 1


def _pad3(tile):
    """[128, NPAD] pad tile viewed as [128, HP, WP]."""
    return tile[:, :].rearrange("p (r c) -> p r c", c=WP)


def _interior(tile, r0=0, nr=H):
    """interior rows r0..r0+nr of the HxW image inside a pad tile."""
    return _pad3(tile)[:, 1 + r0:1 + r0 + nr, 1:1 + W]


def _tap(tile, dy, dx, r0, nr):
    """conv tap read: out rows [r0, r0+nr) <- pad rows [r0+dy, ...)."""
    return _pad3(tile)[:, r0 + dy:r0 + dy + nr, dx:dx + W]


def _memset_border(nc, tile):
    # gpsimd memset rejects float32r in this walrus build; write via a
    # uint32 bitcast (identical bits)
    v = _pad3(tile)
    nc.gpsimd.memset(v[:, 0:1, :].bitcast(U32), 0)
    nc.gpsimd.memset(v[:, HP - 1:HP, :].bitcast(U32), 0)
    nc.gpsimd.memset(v[:, 1:HP - 1, 0:1].bitcast(U32), 0)
    nc.gpsimd.memset(v[:, 1:HP - 1, WP - 1:WP].bitcast(U32), 0)


def build(dt_conv=F32R, dt_attn=F32R, split: bool = True):
    nc = bass.Bass(num_devices=NCORES)

    # ---- DRAM I/O ----
    xp_d = nc.dram_tensor("xp", [BL, KT, 128, NPAD], dt_conv, kind="ExternalInput")
    # ko-major conv weights: [ci][128(i), (ko*9 + tap)*KT + ki -> 128(o)]
    cw_d = nc.dram_tensor("cw", [3, 128, KT * KCOLS], dt_conv, kind="ExternalInput")
    w1t_d = nc.dram_tensor("w1t", [KT, 128, T], F32R, kind="ExternalInput")
    w2t_d = nc.dram_tensor("w2t", [KT, 128, C], F32R, kind="ExternalInput")
    # packed per-channel constants: cols 0-5 conv biases (ci*2+k),
    # 6-9 bn gammas (i*2+k), 10-13 bn betas, 14-15 b_t1, 16-17 b_t2,
    # 18-21 t^T per-core slices (k*BL+s)
    consts_d = nc.dram_tensor("consts", [128, 22], F32R, kind="ExternalInput")
    wqt_d = nc.dram_tensor("wqt", [KT, 128, CQ], dt_attn, kind="ExternalInput")
    wkt_d = nc.dram_tensor("wkt", [KT, 128, CQ], dt_attn, kind="ExternalInput")
    wvt_d = nc.dram_tensor("wvt", [KT, 128, C], dt_attn, kind="ExternalInput")
    bq_d = nc.dram_tensor("bq", [CQ, 1], F32R, kind="ExternalInput")
    bk_d = nc.dram_tensor("bk", [CQ, 1], F32R, kind="ExternalInput")
    bv_d = nc.dram_tensor("bv", [1, C], dt_attn, kind="ExternalInput")
    gam_d = nc.dram_tensor("gam", [1, 1], F32, kind="ExternalInput")
    out_d = nc.dram_tensor("out", [BL, KT, 128, N], F32R, kind="ExternalOutput")

    # collective bounce buffers, one pair per (bn, ko) chunk.
    # AllGather + local DVE reduce beats AllReduce (1.875x modeled cost).
    cc_in = [nc.dram_tensor(f"cc{i}_in", [128, 2], F32) for i in range(4)]
    cc_out = [nc.dram_tensor(f"cc{i}_out", [NCORES, 128, 2], F32,
                             addr_space="Shared") for i in range(4)]

    with TileContext(nc) as tc:
        with (
            tc.tile_pool(name="pconst", bufs=1) as pc,
            tc.tile_pool(name="pcw", bufs=3) as pcw,
            tc.tile_pool(name="ppad", bufs=8) as ppad,
            tc.tile_pool(name="py", bufs=4) as py,
            tc.tile_pool(name="psq", bufs=2) as psq,
            tc.tile_pool(name="pattn", bufs=1) as pat,
            tc.tile_pool(name="pstats", bufs=1) as pst,
            tc.tile_pool(name="ppsum", bufs=8, space="PSUM") as pps,
        ):
            def psum(nm):
                return pps.tile([128, 512], F32, tag="ps", name=nm)

            # ---- big DMAs: x first (conv1 needs it soonest), then conv1
            # weights in ko-major halves, then the rest ----
            cw_sb = [pcw.tile([128, KT * KCOLS], dt_conv, tag="cw", name=f"cw{ci}")
                     for ci in range(3)]
            x_pad = [[ppad.tile([128, NPAD], dt_conv, tag="pad", name=f"xp{s}{k}")
                      for k in range(KT)] for s in range(BL)]
            for k in range(KT):
                nc.sync.dma_start(out=x_pad[0][k][:, :], in_=xp_d[0, k, :, :])
            nc.sync.dma_start(out=cw_sb[0][:, 0:KCOLS], in_=cw_d[0, :, 0:KCOLS])
            for k in range(KT):
                nc.sync.dma_start(out=x_pad[1][k][:, :], in_=xp_d[1, k, :, :])
            nc.sync.dma_start(out=cw_sb[0][:, KCOLS:2 * KCOLS],
                              in_=cw_d[0, :, KCOLS:2 * KCOLS])

            # ---- persistent small tiles ----
            w1t_sb = [pc.tile([128, T], F32R, name=f"w1t{k}") for k in range(KT)]
            w2t_sb = [pc.tile([128, C], F32R, name=f"w2t{k}") for k in range(KT)]
            consts_sb = pc.tile([128, 22], F32R, name="consts_sb")

            def ccol(j, n=1):
                return consts_sb[:, j:j + n]

            cb_sb = [[ccol(ci * KT + k) for k in range(KT)] for ci in range(3)]
            bng_sb = [[ccol(6 + i * KT + k) for k in range(KT)] for i in range(2)]
            bnb_sb = [[ccol(10 + i * KT + k) for k in range(KT)] for i in range(2)]
            bt1_sb = [ccol(14 + k) for k in range(KT)]
            bt2_sb = [ccol(16 + k) for k in range(KT)]
            tt_sb = [ccol(18 + k * BL, BL) for k in range(KT)]
            wqt_sb = [pc.tile([128, CQ], dt_attn, name=f"wqt{k}") for k in range(KT)]
            wkt_sb = [pc.tile([128, CQ], dt_attn, name=f"wkt{k}") for k in range(KT)]
            wvt_sb = [pc.tile([128, C], dt_attn, name=f"wvt{k}") for k in range(KT)]
            bq_sb = pc.tile([CQ, 1], F32R, name="bq_sb")
            bk_sb = pc.tile([CQ, 1], F32R, name="bk_sb")
            bv_sb = pc.tile([1, C], dt_attn, name="bv_sb")
            gam_sb = pc.tile([1, 1], F32, name="gam_sb")
            ones_col = pc.tile([128, 1], dt_attn, name="ones_col")
            ones_row = pc.tile([1, 128], dt_attn, name="ones_row")

            nc.gpsimd.dma_start(out=consts_sb[:, :], in_=consts_d[:, :])
            for k in range(KT):
                nc.sync.dma_start(out=w1t_sb[k][:, :], in_=w1t_d[k, :, :])
                nc.sync.dma_start(out=w2t_sb[k][:, :], in_=w2t_d[k, :, :])
                nc.sync.dma_start(out=wqt_sb[k][:, :], in_=wqt_d[k, :, :])
                nc.sync.dma_start(out=wkt_sb[k][:, :], in_=wkt_d[k, :, :])
                nc.sync.dma_start(out=wvt_sb[k][:, :], in_=wvt_d[k, :, :])
            nc.gpsimd.dma_start(out=bq_sb[:, :], in_=bq_d[:, :])
            nc.gpsimd.dma_start(out=bk_sb[:, :], in_=bk_d[:, :])
            nc.gpsimd.dma_start(out=bv_sb[:, :], in_=bv_d[:, :])
            nc.gpsimd.dma_start(out=gam_sb[:, :], in_=gam_d[:, :])
            nc.gpsimd.memset(ones_col[:, :].bitcast(U32), ONE_F32_BITS)
            nc.gpsimd.memset(ones_row[:, :].bitcast(U32), ONE_F32_BITS)
            # conv2/conv3 weights stream in behind conv1's
            for ci in range(1, 3):
                nc.sync.dma_start(out=cw_sb[ci][:, :], in_=cw_d[ci, :, :])

            # stats: cols [0:8]=sum(ko,s,half), [8:16]=sumsq(ko,s,half)
            stats = [pst.tile([128, 16], F32, name=f"stats{i}") for i in range(2)]
            ccp = [pst.tile([128, 2], F32, name=f"ccp{i}") for i in range(4)]
            glob = [pst.tile([128, 2], F32, name=f"glob{i}") for i in range(4)]
            gall = [pst.tile([128, 2 * NCORES], F32, name=f"gall{i}")
                    for i in range(4)]
            for i in range(2):
                nc.gpsimd.memset(stats[i][:, :], 0.0)

            h1_pad = [[ppad.tile([128, NPAD], dt_conv, tag="pad", name=f"h1p{s}{k}")
                       for k in range(KT)] for s in range(BL)]
            for s in range(BL):
                for k in range(KT):
                    _memset_border(nc, h1_pad[s][k])

            # ---- helpers ----
            def conv_part(ci, src_pads, s, ko, half, ki, ps3, first, last):
                """9 tap matmuls for one (psum, ki) pair."""
                r0 = half * RH
                for tap in range(9):
                    dy, dx = divmod(tap, 3)
                    j = (ko * 9 + tap) * KT + ki
                    nc.tensor.matmul(
                        ps3,
                        cw_sb[ci][:, j * 128:(j + 1) * 128],
                        _tap(src_pads[s][ki], dy, dx, r0, RH),
                        start=(first and tap == 0), stop=(last and tap == 8),
                    )

            def epilogue_stats(i, dst_pad, s, ko, half, ps, ps3):
                """relu(+bias) -> dst interior; sum+sumsq accumulated."""
                col = ko * 4 + s * 2 + half
                nc.scalar.activation(
                    _interior(dst_pad[s][ko], half * RH, RH), ps3, AF.Relu,
                    bias=cb_sb[i][ko][:, :],
                    accum_out=stats[i][:, col:col + 1],
                )
                sq = psq.tile([128, 512], F32, tag="sq", name=f"sq{i}_{s}{ko}{half}")
                nc.scalar.activation(
                    sq[:, :].rearrange("p (r c) -> p r c", c=W),
                    _interior(dst_pad[s][ko], half * RH, RH), AF.Square,
                    accum_out=stats[i][:, 8 + col:9 + col],
                )

            def chunk_sync(i, ko):
                """reduce local (sum, sumsq) for one 128-channel chunk and
                launch its AllGather; returns chunk index."""
                ci_ = i * KT + ko
                nc.vector.reduce_sum(ccp[ci_][:, 0:1],
                                     stats[i][:, ko * 4:ko * 4 + 4], axis=AX.X)
                nc.vector.reduce_sum(ccp[ci_][:, 1:2],
                                     stats[i][:, 8 + ko * 4:12 + ko * 4], axis=AX.X)
                d1 = nc.gpsimd.dma_start(out=cc_in[ci_][:, :], in_=ccp[ci_][:, :])
                cc = nc.gpsimd.collective_compute(
                    "AllGather", ALU.bypass,
                    replica_groups=[list(range(NCORES))],
                    ins=[cc_in[ci_][:].opt()], outs=[cc_out[ci_][:].opt()],
                )
                add_dep_helper(cc.ins, d1.ins, reason="cc waits on stats dma")
                d2 = nc.gpsimd.dma_start(
                    out=gall[ci_][:, :],
                    in_=cc_out[ci_][:, :, :].rearrange("c p k -> p c k"))
                add_dep_helper(d2.ins, cc.ins, reason="readback waits on cc")
                nc.vector.reduce_sum(
                    glob[ci_][:, :],
                    gall[ci_][:, :].rearrange("p (c k) -> p k c", k=2), axis=AX.X)
                return ci_

            def bn_consts(ci_, i, ko):
                """per-chunk (scale, shift) from glob[ci_]."""
                mean = pst.tile([128, 1], F32, name=f"mean{ci_}")
                ex2 = pst.tile([128, 1], F32, name=f"ex2{ci_}")
                var = pst.tile([128, 1], F32, name=f"var{ci_}")
                rv = pst.tile([128, 1], F32, name=f"rv{ci_}")
                scl = pst.tile([128, 1], F32, name=f"scl{ci_}")
                shf = pst.tile([128, 1], F32, name=f"shf{ci_}")
                nc.vector.tensor_scalar_mul(mean[:, :], glob[ci_][:, 0:1], 1.0 / NPIX)
                nc.vector.tensor_scalar_mul(ex2[:, :], glob[ci_][:, 1:2], 1.0 / NPIX)
                nc.vector.tensor_tensor(var[:, :], mean[:, :], mean[:, :], ALU.mult)
                nc.vector.tensor_tensor(var[:, :], ex2[:, :], var[:, :], ALU.subtract)
                nc.vector.tensor_scalar(out=var[:, :], in0=var[:, :], scalar1=EPS,
                                        scalar2=None, op0=ALU.add)
                nc.vector.reciprocal(rv[:, :], var[:, :])
                nc.scalar.activation(rv[:, :], rv[:, :], AF.Sqrt)
                # Newton step: y' = 0.5*y*(3 - v*y^2) for accuracy
                t1 = pst.tile([128, 1], F32, name=f"nr{ci_}")
                nc.vector.tensor_tensor(t1[:, :], rv[:, :], rv[:, :], ALU.mult)
                nc.vector.tensor_tensor(t1[:, :], var[:, :], t1[:, :], ALU.mult)
                nc.vector.tensor_scalar(out=t1[:, :], in0=t1[:, :], scalar1=-1.0,
                                        scalar2=3.0, op0=ALU.mult, op1=ALU.add)
                nc.vector.tensor_tensor(t1[:, :], rv[:, :], t1[:, :], ALU.mult)
                nc.vector.tensor_scalar_mul(rv[:, :], t1[:, :], 0.5)
                nc.vector.tensor_tensor(scl[:, :], rv[:, :], bng_sb[i][ko][:, :],
                                        ALU.mult)
                nc.vector.tensor_tensor(shf[:, :], mean[:, :], scl[:, :], ALU.mult)
                nc.vector.tensor_tensor(shf[:, :], bnb_sb[i][ko][:, :], shf[:, :],
                                        ALU.subtract)
                return scl, shf

            # ================= conv1, chunked by ko =================
            for ko in range(KT):
                for s in range(BL):
                    for half in range(2):
                        ps = psum(f"c1_{s}{ko}{half}")
                        ps3 = ps[:, :].rearrange("p (r c) -> p r c", c=W)
                        for ki in range(KT):
                            conv_part(0, x_pad, s, ko, half, ki, ps3,
                                      first=(ki == 0), last=(ki == KT - 1))
                        epilogue_stats(0, h1_pad, s, ko, half, ps, ps3)
                chunk_sync(0, ko)

            # ---- time MLP (PE queue slot: behind conv1, overlaps cc) ----
            te1_sb = [pst.tile([128, BL], F32R, name=f"te1_{m}") for m in range(KT)]
            te_sb = [pst.tile([128, BL], F32R, name=f"te_{m}") for m in range(KT)]
            for mo in range(KT):
                ps = psum(f"mlp1_{mo}")
                for ki in range(KT):
                    nc.tensor.matmul(ps[:, 0:BL],
                                     w1t_sb[ki][:, mo * 128:(mo + 1) * 128],
                                     tt_sb[ki][:, :],
                                     start=(ki == 0), stop=(ki == KT - 1))
                nc.scalar.activation(te1_sb[mo][:, :], ps[:, 0:BL], AF.Relu,
                                     bias=bt1_sb[mo][:, :])
            for mo in range(KT):
                ps = psum(f"mlp2_{mo}")
                for ki in range(KT):
                    nc.tensor.matmul(ps[:, 0:BL],
                                     w2t_sb[ki][:, mo * 128:(mo + 1) * 128],
                                     te1_sb[ki][:, :],
                                     start=(ki == 0), stop=(ki == KT - 1))
                nc.scalar.activation(te_sb[mo][:, :], ps[:, 0:BL], AF.Relu,
                                     bias=bt2_sb[mo][:, :])

            # ================= BN1-ko0 -> conv2 ki0 phase =================
            scl0, shf0 = bn_consts(0, 0, 0)
            for s in range(BL):
                bsk = pst.tile([128, 1], F32, name=f"b1s{s}0")
                nc.vector.tensor_tensor(bsk[:, :], shf0[:, :],
                                        te_sb[0][:, s:s + 1], ALU.add)
                nc.scalar.activation(_interior(h1_pad[s][0]),
                                     _interior(h1_pad[s][0]), AF.Identity,
                                     bias=bsk[:, :], scale=scl0[:, :])

            c2ps = {}
            for s in range(BL):
                for ko in range(KT):
                    for half in range(2):
                        ps = psum(f"c2_{s}{ko}{half}")
                        c2ps[(s, ko, half)] = ps
                        ps3 = ps[:, :].rearrange("p (r c) -> p r c", c=W)
                        conv_part(1, h1_pad, s, ko, half, 0, ps3,
                                  first=True, last=False)

            # ---- BN1-ko1 -> normalize -> conv2 ki1 (ko-ordered) ----
            scl1, shf1 = bn_consts(1, 0, 1)
            for s in range(BL):
                bsk = pst.tile([128, 1], F32, name=f"b1s{s}1")
                nc.vector.tensor_tensor(bsk[:, :], shf1[:, :],
                                        te_sb[1][:, s:s + 1], ALU.add)
                nc.scalar.activation(_interior(h1_pad[s][1]),
                                     _interior(h1_pad[s][1]), AF.Identity,
                                     bias=bsk[:, :], scale=scl1[:, :])

            h2_pad = [[ppad.tile([128, NPAD], dt_conv, tag="pad", name=f"h2p{s}{k}")
                       for k in range(KT)] for s in range(BL)]
            for s in range(BL):
                for k in range(KT):
                    _memset_border(nc, h2_pad[s][k])

            for ko in range(KT):
                for s in range(BL):
                    for half in range(2):
                        ps = c2ps[(s, ko, half)]
                        ps3 = ps[:, :].rearrange("p (r c) -> p r c", c=W)
                        conv_part(1, h1_pad, s, ko, half, 1, ps3,
                                  first=False, last=True)
                        epilogue_stats(1, h2_pad, s, ko, half, ps, ps3)
                chunk_sync(1, ko)

            # ================= BN2-ko0 -> conv3 ki0 phase =================
            scl2, shf2 = bn_consts(2, 1, 0)
            for s in range(BL):
                nc.scalar.activation(_interior(h2_pad[s][0]),
                                     _interior(h2_pad[s][0]), AF.Identity,
                                     bias=shf2[:, :], scale=scl2[:, :])

            c3ps = {}
            for s in range(BL):
                for ko in range(KT):
                    for half in range(2):
                        ps = psum(f"c3_{s}{ko}{half}")
                        c3ps[(s, ko, half)] = ps
                        ps3 = ps[:, :].rearrange("p (r c) -> p r c", c=W)
                        conv_part(2, h2_pad, s, ko, half, 0, ps3,
                                  first=True, last=False)

            scl3, shf3 = bn_consts(3, 1, 1)
            for s in range(BL):
                nc.scalar.activation(_interior(h2_pad[s][1]),
                                     _interior(h2_pad[s][1]), AF.Identity,
                                     bias=shf3[:, :], scale=scl3[:, :])

            y_sb = [[py.tile([128, N], dt_attn, tag="y", name=f"y{s}{k}")
                     for k in range(KT)] for s in range(BL)]
            for s in range(BL):
                for ko in range(KT):
                    for half in range(2):
                        ps = c3ps[(s, ko, half)]
                        ps3 = ps[:, :].rearrange("p (r c) -> p r c", c=W)
                        conv_part(2, h2_pad, s, ko, half, 1, ps3,
                                  first=False, last=True)
                        nc.scalar.activation(
                            y_sb[s][ko][:, half * 512:(half + 1) * 512],
                            ps[:, :], AF.Identity, bias=cb_sb[2][ko][:, :])

            # ================= attention (per sample) =================
            for s in range(BL):
                # V^T tiles: [n-tile 128, C]
                vt = []
                for nt in range(8):
                    ps = psum(f"vps{s}{nt}")
                    pv = ps[:, 0:C]
                    for c2 in range(KT):
                        nc.tensor.matmul(pv, y_sb[s][c2][:, nt * 128:(nt + 1) * 128],
                                         wvt_sb[c2][:, :], start=(c2 == 0), stop=False)
                    nc.tensor.matmul(pv, ones_row[:, :], bv_sb[:, :],
                                     start=False, stop=True)
                    v = pat.tile([128, C], dt_attn, tag="vt", bufs=9, name=f"vt{s}{nt}")
                    nc.vector.tensor_copy(v[:, :], pv)
                    vt.append(v)

                # Q, K: [CQ, N]
                q_sb = pat.tile([CQ, N], dt_attn, tag="q", bufs=2, name=f"q{s}")
                k_sb = pat.tile([CQ, N], dt_attn, tag="k", bufs=2, name=f"k{s}")
                for nh in range(2):
                    psq_ = psum(f"qps{s}{nh}")
                    for c2 in range(KT):
                        nc.tensor.matmul(psq_[0:CQ, :], wqt_sb[c2][:, :],
                                         y_sb[s][c2][:, nh * 512:(nh + 1) * 512],
                                         start=(c2 == 0), stop=(c2 == KT - 1))
                    nc.scalar.activation(q_sb[:, nh * 512:(nh + 1) * 512],
                                         psq_[0:CQ, :], AF.Identity, bias=bq_sb[:, :])
                    psk_ = psum(f"kps{s}{nh}")
                    for c2 in range(KT):
                        nc.tensor.matmul(psk_[0:CQ, :], wkt_sb[c2][:, :],
                                         y_sb[s][c2][:, nh * 512:(nh + 1) * 512],
                                         start=(c2 == 0), stop=(c2 == KT - 1))
                    nc.scalar.activation(k_sb[:, nh * 512:(nh + 1) * 512],
                                         psk_[0:CQ, :], AF.Identity, bias=bk_sb[:, :])

                res_t = [pat.tile([128, N], F32R, tag="res", bufs=4,
                                  name=f"res{s}{c2}") for c2 in range(KT)]
                for nh in range(2):
                    # S^T tiles -> P = exp(S^T)  (no max-shift: |logits| << 80)
                    ptiles = []
                    for mt in range(8):
                        ps = psum(f"sps{s}{nh}{mt}")
                        nc.tensor.matmul(ps[:, :], k_sb[:, mt * 128:(mt + 1) * 128],
                                         q_sb[:, nh * 512:(nh + 1) * 512],
                                         start=True, stop=True)
                        p = pat.tile([128, 512], dt_attn, tag="P", bufs=16,
                                     name=f"P{s}{nh}{mt}")
                        nc.scalar.activation(p[:, :], ps[:, :], AF.Exp)
                        ptiles.append(p)
                    # V @ P output matmuls FIRST in PE order (PE never waits
                    # on the denominator chain)
                    pr_ps = []
                    for c2 in range(KT):
                        pr = psum(f"rps{s}{nh}{c2}")
                        pr_ps.append(pr)
                        for mt in range(8):
                            nc.tensor.matmul(pr[:, :],
                                             vt[mt][:, c2 * 128:(c2 + 1) * 128],
                                             ptiles[mt][:, :],
                                             start=(mt == 0), stop=(mt == 7))
                    # denom[n] = sum_m P: DVE add-tree (overlaps the pr
                    # matmuls), then one ones-matmul for the partition reduce
                    pacc = pat.tile([128, 512], dt_attn, tag="pacc", bufs=2,
                                    name=f"pacc{s}{nh}")
                    nc.vector.tensor_tensor(pacc[:, :], ptiles[0][:, :],
                                            ptiles[1][:, :], ALU.add)
                    for mt in range(2, 8):
                        nc.vector.tensor_tensor(pacc[:, :], pacc[:, :],
                                                ptiles[mt][:, :], ALU.add)
                    pd = psum(f"dps{s}{nh}")
                    nc.tensor.matmul(pd[0:1, :], ones_col[:, :], pacc[:, :],
                                     start=True, stop=True)
                    rcp = pat.tile([1, 512], dt_attn, tag="rcp", bufs=2,
                                   name=f"rcp{s}{nh}")
                    with nc.allow_low_precision(reason="f32r==f32 bit layout"):
                        nc.vector.reciprocal(rcp[:, :], pd[0:1, :])
                    nc.vector.tensor_scalar(out=rcp[:, :], in0=rcp[:, :],
                                            scalar1=gam_sb[0:1, 0:1], scalar2=None,
                                            op0=ALU.mult)
                    # broadcast gamma/denom down partitions
                    pb = psum(f"bps{s}{nh}")
                    nc.tensor.matmul(pb[:, :], ones_row[:, :], rcp[:, :],
                                     start=True, stop=True)
                    rb = pat.tile([128, 512], F32, tag="rb", bufs=2, name=f"rb{s}{nh}")
                    nc.vector.tensor_copy(rb[:, :], pb[:, :])
                    # out = (V @ P) * rb + y ; stream each half to HBM
                    for c2 in range(KT):
                        rs = res_t[c2][:, nh * 512:(nh + 1) * 512]
                        nc.vector.tensor_tensor(rs, pr_ps[c2][:, :], rb[:, :],
                                                ALU.mult)
                        nc.vector.tensor_tensor(rs, rs,
                                                y_sb[s][c2][:, nh * 512:(nh + 1) * 512],
                                                ALU.add)
                        nc.sync.dma_start(
                            out=out_d[s, c2, :, nh * 512:(nh + 1) * 512],
                            in_=rs)

    if split:
        _split_packed_waits(nc)
    return nc


def _prep_inputs(inputs):
    """host-side reshape/transpose; returns per-core input maps"""
    f32 = np.float32
    x = np.asarray(inputs["x"], f32)
    t = np.asarray(inputs["t"], f32)

    def conv_w(w):
        w6 = np.asarray(w, f32).reshape(KT, 128, KT, 128, 3, 3)  # ko,o,ki,i,dy,dx
        arr = w6.transpose(3, 0, 4, 5, 2, 1)  # i,ko,dy,dx,ki,o
        return np.ascontiguousarray(arr.reshape(128, KT * KCOLS))

    cw = np.stack([conv_w(inputs["w_c1"]), conv_w(inputs["w_c2"]),
                   conv_w(inputs["w_tr"])])
    w1t = np.ascontiguousarray(np.asarray(inputs["w_t1"], f32).T.reshape(KT, 128, T))
    w2t = np.ascontiguousarray(np.asarray(inputs["w_t2"], f32).T.reshape(KT, 128, C))
    # packed per-channel constants (see consts_d layout in build())
    consts = np.zeros((128, 22), f32)
    for ci, k2 in enumerate(("b_c1", "b_c2", "b_tr")):
        consts[:, ci * KT:(ci + 1) * KT] = np.asarray(inputs[k2], f32).reshape(KT, 128).T
    for i, (gk, bk2) in enumerate((("bn1_g", "bn1_b"), ("bn2_g", "bn2_b"))):
        consts[:, 6 + i * KT:6 + (i + 1) * KT] = np.asarray(inputs[gk], f32).reshape(KT, 128).T
        consts[:, 10 + i * KT:10 + (i + 1) * KT] = np.asarray(inputs[bk2], f32).reshape(KT, 128).T
    consts[:, 14:16] = np.asarray(inputs["b_t1"], f32).reshape(KT, 128).T
    consts[:, 16:18] = np.asarray(inputs["b_t2"], f32).reshape(KT, 128).T
    wqt = np.ascontiguousarray(np.asarray(inputs["wq"], f32).T.reshape(KT, 128, CQ))
    wkt = np.ascontiguousarray(np.asarray(inputs["wk"], f32).T.reshape(KT, 128, CQ))
    wvt = np.ascontiguousarray(np.asarray(inputs["wv"], f32).T.reshape(KT, 128, C))
    bq = np.asarray(inputs["bq"], f32).reshape(CQ, 1)
    bk = np.asarray(inputs["bk"], f32).reshape(CQ, 1)
    bv = np.asarray(inputs["bv"], f32).reshape(1, C)
    gam = np.asarray(inputs["gamma"], f32).reshape(1, 1)

    xp = np.zeros((B, KT, 128, HP, WP), f32)
    xp[:, :, :, 1:1 + H, 1:1 + W] = x.reshape(B, KT, 128, H, W)
    xp = xp.reshape(B, KT, 128, NPAD)
    ttr = np.ascontiguousarray(t.T.reshape(KT, 128, B))

    shared = dict(cw=cw, w1t=w1t, w2t=w2t,
                  wqt=wqt, wkt=wkt, wvt=wvt, bq=bq, bk=bk, bv=bv, gam=gam)
    per_core = []
    for c in range(NCORES):
        m = dict(shared)
        m["xp"] = np.ascontiguousarray(xp[c * BL:(c + 1) * BL])
        cc_consts = consts.copy()
        for k in range(KT):
            cc_consts[:, 18 + k * BL:18 + (k + 1) * BL] = \
                ttr[k, :, c * BL:(c + 1) * BL]
        m["consts"] = cc_consts
        per_core.append(m)
    return per_core


def _unshard(results):
    out = np.empty((B, C, H, W), np.float32)
    for c in range(NCORES):
        o = results[c]["out"].reshape(BL, KT, 128, H, W)
        for s in range(BL):
            out[c * BL + s] = o[s].reshape(C, H, W)
    return out


_cache = {}

DT_CONV = F32R
DT_ATTN = F32R


def kernel(**inputs) -> np.ndarray:
    key = ("nc", str(DT_CONV), str(DT_ATTN))
    if key not in _cache:
        _cache[key] = build(dt_conv=DT_CONV, dt_attn=DT_ATTN)
    nc = _cache[key]
    per_core = _prep_inputs(inputs)
    try:
        res = run_bass_kernel_spmd(nc, per_core, core_ids=list(range(NCORES)))
    except Exception:
        # transient NRT_EXEC_UNIT_UNRECOVERABLE errors recover on re-execute
        res = run_bass_kernel_spmd(nc, per_core, core_ids=list(range(NCORES)))
    return _unshard(res.results)


# revision 4
# speedup vs baseline: 1.3121x; 1.1022x over previous
"""Trainium2 Bass kernel for nn_BlockWithAttention (dense CNN block + attention).

Sharding: data-parallel over batch (B=16 -> 2 samples/core x 8 cores).

Scheduling design (tuned against the TimelineSim cost model):
- The PE pstate ramp model freezes each matmul's clock at cost time, so any
  PE idle gap poisons the following flood with 2-3.7x slower matmuls. Tiny
  "filler" matmuls ([128,32] ones, dedicated PSUM bank) bridge every
  potential idle window so real matmuls always cost at the 2.4GHz peak.
- BN batch-stat sync is chunked per 128-channel group (2 AllGathers per BN)
  and pipelined: BN1-ko0's collective flies during conv1-ko1, BN1-ko1's
  during conv2's ki0 phase, BN2's two during conv2-ki1/conv3-ki0.
- conv2/conv3 split into ki phases with 7 psums held open across the phase
  boundary (the 8th PSUM bank belongs to the fillers); the 8th conv psum
  runs as a plain 18-chain in the ki1 phase.
- attention: V@P output matmuls are emitted before the softmax-denominator
  reduce; the denominator add-tree is split in two for a shorter tail;
  gamma is folded into wv/bv host-side.

All matmuls run in float32r (full PE rate at free-size>=256); accumulation
is fp32 in PSUM.
"""
import numpy as np

import concourse.bass as bass
import concourse.mybir as mybir
from concourse.bass_utils import run_bass_kernel_spmd
from concourse.tile import TileContext
from concourse.tile_rust import add_dep_helper

# ---- problem constants ----
B, C, H, W, T, CQ = 16, 256, 32, 32, 256, 32
NCORES = 8
BL = B // NCORES            # samples per core
KT = C // 128               # 128-channel tiles
HP, WP = H + 2, W + 2       # padded image
NPAD = HP * WP              # 1156
NPIX = B * H * W            # BN stat count (full batch)
N = H * W                   # 1024 spatial positions
RH = 16                     # rows per 512-px half
EPS = 1e-5
KCOLS = 9 * KT * 128        # weight cols per ko group (ko-major layout)

F32 = mybir.dt.float32
F32R = mybir.dt.float32r
AX = mybir.AxisListType
ALU = mybir.AluOpType
AF = mybir.ActivationFunctionType

U32 = mybir.dt.uint32
ONE_F32_BITS = 0x3F800000

# filler-block sizes (tuned against TimelineSim)
FILL = {
    "ign0": 130,   # kernel start -> first conv1 matmul
    "A": 60,       # conv1/MLP -> conv2-ki0 (BN1-ko0 wait)
    "B": 15,       # conv2-ki0 -> conv2-ki1 (BN1-ko1 wait, insurance)
    "C": 190,      # conv2 end -> conv3-ki0 (BN2-ko0 collective wait)
    "D": 15,       # conv3-ki0 -> conv3-ki1 (insurance)
    "E": 25,       # conv3 -> attention V (y epilogue trail)
    "V": 20,       # per-sample V block lead-in
    "S": 20,       # per-(s,nh) S block lead-in
}

_wsplit_counter = [0]


def _split_packed_waits(nc, max_waits: int = 1):
    """The walrus build here rejects >1-2 packed sync-waits per instruction
    ("Too many sync wait commands"). Move excess waits onto standalone
    single-wait EventSemaphore carriers inserted before the instruction
    (same engine -> program order preserves gating)."""
    for f in nc.m.functions:
        for bb in f.blocks:
            il = bb.instructions
            i = 0
            while i < len(il):
                inst = il[i]
                si = inst.sync_info
                if si is not None and len(si.on_wait) > max_waits:
                    waits = list(si.on_wait)
                    movable = [w for w in waits if w.wait_reg is None]
                    fixed = [w for w in waits if w.wait_reg is not None]
                    keep_n = max(0, max_waits - len(fixed))
                    kept = fixed + movable[:keep_n]
                    move = movable[keep_n:]
                    if not move:
                        i += 1
                        continue
                    si.on_wait = kept
                    for w in move:
                        _wsplit_counter[0] += 1
                        ev = mybir.InstEventSemaphore(
                            name=f"I-wsplit-{_wsplit_counter[0]}",
                            opcode="EventSemaphore",
                            engine=inst.engine,
                            sync_info=mybir.SyncInfo(on_wait=[w], on_update=[]),
                        )
                        il.insert(i, ev)
                        i += 1
                i += 1


def _pad3(tile):
    return tile[:, :].rearrange("p (r c) -> p r c", c=WP)


def _interior(tile, r0=0, nr=H):
    return _pad3(tile)[:, 1 + r0:1 + r0 + nr, 1:1 + W]


def _tap(tile, dy, dx, r0, nr):
    return _pad3(tile)[:, r0 + dy:r0 + dy + nr, dx:dx + W]


def _memset_border(nc, tile):
    v = _pad3(tile)
    nc.gpsimd.memset(v[:, 0:1, :].bitcast(U32), 0)
    nc.gpsimd.memset(v[:, HP - 1:HP, :].bitcast(U32), 0)
    nc.gpsimd.memset(v[:, 1:HP - 1, 0:1].bitcast(U32), 0)
    nc.gpsimd.memset(v[:, 1:HP - 1, WP - 1:WP].bitcast(U32), 0)


def build(dt_conv=F32R, dt_attn=F32R, split: bool = True, fill=None):
    fill = dict(FILL, **(fill or {}))
    nc = bass.Bass(num_devices=NCORES)

    # ---- DRAM I/O ----
    xp_d = nc.dram_tensor("xp", [BL, KT, 128, NPAD], dt_conv, kind="ExternalInput")
    # ko-major conv weights: [ci][128(i), ((ko*9 + tap)*KT + ki)*128 + o]
    cw_d = nc.dram_tensor("cw", [3, 128, KT * KCOLS], dt_conv, kind="ExternalInput")
    w1t_d = nc.dram_tensor("w1t", [KT, 128, T], F32R, kind="ExternalInput")
    w2t_d = nc.dram_tensor("w2t", [KT, 128, C], F32R, kind="ExternalInput")
    consts_d = nc.dram_tensor("consts", [128, 22], F32R, kind="ExternalInput")
    wqt_d = nc.dram_tensor("wqt", [KT, 128, CQ], dt_attn, kind="ExternalInput")
    wkt_d = nc.dram_tensor("wkt", [KT, 128, CQ], dt_attn, kind="ExternalInput")
    wvt_d = nc.dram_tensor("wvt", [KT, 128, C], dt_attn, kind="ExternalInput")
    bq_d = nc.dram_tensor("bq", [CQ, 1], F32R, kind="ExternalInput")
    bk_d = nc.dram_tensor("bk", [CQ, 1], F32R, kind="ExternalInput")
    bv_d = nc.dram_tensor("bv", [1, C], dt_attn, kind="ExternalInput")
    out_d = nc.dram_tensor("out", [BL, KT, 128, N], F32R, kind="ExternalOutput")

    cc_in = [nc.dram_tensor(f"cc{i}_in", [128, 2], F32) for i in range(4)]
    cc_out = [nc.dram_tensor(f"cc{i}_out", [NCORES, 128, 2], F32,
                             addr_space="Shared") for i in range(4)]

    with TileContext(nc) as tc:
        with (
            tc.tile_pool(name="pconst", bufs=1) as pc,
            tc.tile_pool(name="pcw", bufs=3) as pcw,
            tc.tile_pool(name="ppad", bufs=8) as ppad,
            tc.tile_pool(name="py", bufs=4) as py,
            tc.tile_pool(name="psq", bufs=2) as psq,
            tc.tile_pool(name="pattn", bufs=1) as pat,
            tc.tile_pool(name="pstats", bufs=1) as pst,
            tc.tile_pool(name="ppsum", bufs=1, space="PSUM") as pps,
        ):
            def psum(nm):
                return pps.tile([128, 512], F32, tag="ps", bufs=7, name=nm)

            # ---- filler infrastructure: dedicated PSUM bank + ones tile ----
            fones = pc.tile([128, 32], F32R, name="fones")
            nc.gpsimd.memset(fones[:, :].bitcast(U32), ONE_F32_BITS)
            fps = pps.tile([32, 512], F32, tag="fill", bufs=1, name="fps")

            def filler(n):
                for _ in range(n):
                    nc.tensor.matmul(fps[0:32, 0:32], fones[:, :], fones[:, :],
                                     start=True, stop=True)

            # ---- big DMAs: x first, conv1 weights in ko-major halves ----
            cw_sb = [pcw.tile([128, KT * KCOLS], dt_conv, tag="cw", name=f"cw{ci}")
                     for ci in range(3)]
            x_pad = [[ppad.tile([128, NPAD], dt_conv, tag="pad", name=f"xp{s}{k}")
                      for k in range(KT)] for s in range(BL)]
            for k in range(KT):
                nc.sync.dma_start(out=x_pad[0][k][:, :], in_=xp_d[0, k, :, :])
            nc.sync.dma_start(out=cw_sb[0][:, 0:KCOLS], in_=cw_d[0, :, 0:KCOLS])
            for k in range(KT):
                nc.sync.dma_start(out=x_pad[1][k][:, :], in_=xp_d[1, k, :, :])
            nc.sync.dma_start(out=cw_sb[0][:, KCOLS:2 * KCOLS],
                              in_=cw_d[0, :, KCOLS:2 * KCOLS])

            # ---- persistent small tiles ----
            w1t_sb = [pc.tile([128, T], F32R, name=f"w1t{k}") for k in range(KT)]
            w2t_sb = [pc.tile([128, C], F32R, name=f"w2t{k}") for k in range(KT)]
            consts_sb = pc.tile([128, 22], F32R, name="consts_sb")

            def ccol(j, n=1):
                return consts_sb[:, j:j + n]

            cb_sb = [[ccol(ci * KT + k) for k in range(KT)] for ci in range(3)]
            bng_sb = [[ccol(6 + i * KT + k) for k in range(KT)] for i in range(2)]
            bnb_sb = [[ccol(10 + i * KT + k) for k in range(KT)] for i in range(2)]
            bt1_sb = [ccol(14 + k) for k in range(KT)]
            bt2_sb = [ccol(16 + k) for k in range(KT)]
            tt_sb = [ccol(18 + k * BL, BL) for k in range(KT)]
            wqt_sb = [pc.tile([128, CQ], dt_attn, name=f"wqt{k}") for k in range(KT)]
            wkt_sb = [pc.tile([128, CQ], dt_attn, name=f"wkt{k}") for k in range(KT)]
            wvt_sb = [pc.tile([128, C], dt_attn, name=f"wvt{k}") for k in range(KT)]
            bq_sb = pc.tile([CQ, 1], F32R, name="bq_sb")
            bk_sb = pc.tile([CQ, 1], F32R, name="bk_sb")
            bv_sb = pc.tile([1, C], dt_attn, name="bv_sb")
            ones_col = pc.tile([128, 1], dt_attn, name="ones_col")
            ones_row = pc.tile([1, 128], dt_attn, name="ones_row")

            nc.gpsimd.dma_start(out=consts_sb[:, :], in_=consts_d[:, :])
            for k in range(KT):
                nc.sync.dma_start(out=w1t_sb[k][:, :], in_=w1t_d[k, :, :])
                nc.sync.dma_start(out=w2t_sb[k][:, :], in_=w2t_d[k, :, :])
                nc.sync.dma_start(out=wqt_sb[k][:, :], in_=wqt_d[k, :, :])
                nc.sync.dma_start(out=wkt_sb[k][:, :], in_=wkt_d[k, :, :])
                nc.sync.dma_start(out=wvt_sb[k][:, :], in_=wvt_d[k, :, :])
            nc.gpsimd.dma_start(out=bq_sb[:, :], in_=bq_d[:, :])
            nc.gpsimd.dma_start(out=bk_sb[:, :], in_=bk_d[:, :])
            nc.gpsimd.dma_start(out=bv_sb[:, :], in_=bv_d[:, :])
            nc.gpsimd.memset(ones_col[:, :].bitcast(U32), ONE_F32_BITS)
            nc.gpsimd.memset(ones_row[:, :].bitcast(U32), ONE_F32_BITS)
            for ci in range(1, 3):
                nc.sync.dma_start(out=cw_sb[ci][:, :], in_=cw_d[ci, :, :])

            stats = [pst.tile([128, 16], F32, name=f"stats{i}") for i in range(2)]
            ccp = [pst.tile([128, 2], F32, name=f"ccp{i}") for i in range(4)]
            glob = [pst.tile([128, 2], F32, name=f"glob{i}") for i in range(4)]
            gall = [pst.tile([128, 2 * NCORES], F32, name=f"gall{i}")
                    for i in range(4)]
            for i in range(2):
                nc.gpsimd.memset(stats[i][:, :], 0.0)

            h1_pad = [[ppad.tile([128, NPAD], dt_conv, tag="pad", name=f"h1p{s}{k}")
                       for k in range(KT)] for s in range(BL)]
            for s in range(BL):
                for k in range(KT):
                    _memset_border(nc, h1_pad[s][k])

            # ---- helpers ----
            def conv_part(ci, src_pads, s, ko, half, ki, ps3, first, last):
                r0 = half * RH
                for tap in range(9):
                    dy, dx = divmod(tap, 3)
                    j = (ko * 9 + tap) * KT + ki
                    nc.tensor.matmul(
                        ps3,
                        cw_sb[ci][:, j * 128:(j + 1) * 128],
                        _tap(src_pads[s][ki], dy, dx, r0, RH),
                        start=(first and tap == 0), stop=(last and tap == 8),
                    )

            def epilogue_stats(i, dst_pad, s, ko, half, ps3):
                col = ko * 4 + s * 2 + half
                nc.scalar.activation(
                    _interior(dst_pad[s][ko], half * RH, RH), ps3, AF.Relu,
                    bias=cb_sb[i][ko][:, :],
                    accum_out=stats[i][:, col:col + 1],
                )
                sq = psq.tile([128, 512], F32, tag="sq", name=f"sq{i}_{s}{ko}{half}")
                nc.scalar.activation(
                    sq[:, :].rearrange("p (r c) -> p r c", c=W),
                    _interior(dst_pad[s][ko], half * RH, RH), AF.Square,
                    accum_out=stats[i][:, 8 + col:9 + col],
                )

            def chunk_sync(i, ko):
                ci_ = i * KT + ko
                nc.vector.reduce_sum(ccp[ci_][:, 0:1],
                                     stats[i][:, ko * 4:ko * 4 + 4], axis=AX.X)
                nc.vector.reduce_sum(ccp[ci_][:, 1:2],
                                     stats[i][:, 8 + ko * 4:12 + ko * 4], axis=AX.X)
                d1 = nc.gpsimd.dma_start(out=cc_in[ci_][:, :], in_=ccp[ci_][:, :])
                cc = nc.gpsimd.collective_compute(
                    "AllGather", ALU.bypass,
                    replica_groups=[list(range(NCORES))],
                    ins=[cc_in[ci_][:].opt()], outs=[cc_out[ci_][:].opt()],
                )
                add_dep_helper(cc.ins, d1.ins, reason="cc waits on stats dma")
                d2 = nc.gpsimd.dma_start(
                    out=gall[ci_][:, :],
                    in_=cc_out[ci_][:, :, :].rearrange("c p k -> p c k"))
                add_dep_helper(d2.ins, cc.ins, reason="readback waits on cc")
                nc.vector.reduce_sum(
                    glob[ci_][:, :],
                    gall[ci_][:, :].rearrange("p (c k) -> p k c", k=2), axis=AX.X)

            def bn_consts(ci_, i, ko):
                mean = pst.tile([128, 1], F32, name=f"mean{ci_}")
                ex2 = pst.tile([128, 1], F32, name=f"ex2{ci_}")
                var = pst.tile([128, 1], F32, name=f"var{ci_}")
                rv = pst.tile([128, 1], F32, name=f"rv{ci_}")
                scl = pst.tile([128, 1], F32, name=f"scl{ci_}")
                shf = pst.tile([128, 1], F32, name=f"shf{ci_}")
                nc.vector.tensor_scalar_mul(mean[:, :], glob[ci_][:, 0:1], 1.0 / NPIX)
                nc.vector.tensor_scalar_mul(ex2[:, :], glob[ci_][:, 1:2], 1.0 / NPIX)
                nc.vector.tensor_tensor(var[:, :], mean[:, :], mean[:, :], ALU.mult)
                nc.vector.tensor_tensor(var[:, :], ex2[:, :], var[:, :], ALU.subtract)
                nc.vector.tensor_scalar(out=var[:, :], in0=var[:, :], scalar1=EPS,
                                        scalar2=None, op0=ALU.add)
                nc.vector.reciprocal(rv[:, :], var[:, :])
                nc.scalar.activation(rv[:, :], rv[:, :], AF.Sqrt)
                # Newton step: y' = 0.5*y*(3 - v*y^2)
                t1 = pst.tile([128, 1], F32, name=f"nr{ci_}")
                nc.vector.tensor_tensor(t1[:, :], rv[:, :], rv[:, :], ALU.mult)
                nc.vector.tensor_tensor(t1[:, :], var[:, :], t1[:, :], ALU.mult)
                nc.vector.tensor_scalar(out=t1[:, :], in0=t1[:, :], scalar1=-1.0,
                                        scalar2=3.0, op0=ALU.mult, op1=ALU.add)
                nc.vector.tensor_tensor(t1[:, :], rv[:, :], t1[:, :], ALU.mult)
                nc.vector.tensor_scalar_mul(rv[:, :], t1[:, :], 0.5)
                nc.vector.tensor_tensor(scl[:, :], rv[:, :], bng_sb[i][ko][:, :],
                                        ALU.mult)
                nc.vector.tensor_tensor(shf[:, :], mean[:, :], scl[:, :], ALU.mult)
                nc.vector.tensor_tensor(shf[:, :], bnb_sb[i][ko][:, :], shf[:, :],
                                        ALU.subtract)
                return scl, shf

            # ================= ignition + conv1 (chunked by ko) ============
            filler(fill["ign0"])
            for ko in range(KT):
                for s in range(BL):
                    for half in range(2):
                        ps = psum(f"c1_{s}{ko}{half}")
                        ps3 = ps[:, :].rearrange("p (r c) -> p r c", c=W)
                        for ki in range(KT):
                            conv_part(0, x_pad, s, ko, half, ki, ps3,
                                      first=(ki == 0), last=(ki == KT - 1))
                        epilogue_stats(0, h1_pad, s, ko, half, ps3)
                chunk_sync(0, ko)

            # ---- time MLP ----
            te1_sb = [pst.tile([128, BL], F32R, name=f"te1_{m}") for m in range(KT)]
            te_sb = [pst.tile([128, BL], F32R, name=f"te_{m}") for m in range(KT)]
            for mo in range(KT):
                ps = psum(f"mlp1_{mo}")
                for ki in range(KT):
                    nc.tensor.matmul(ps[:, 0:BL],
                                     w1t_sb[ki][:, mo * 128:(mo + 1) * 128],
                                     tt_sb[ki][:, :],
                                     start=(ki == 0), stop=(ki == KT - 1))
                nc.scalar.activation(te1_sb[mo][:, :], ps[:, 0:BL], AF.Relu,
                                     bias=bt1_sb[mo][:, :])
            for mo in range(KT):
                ps = psum(f"mlp2_{mo}")
                for ki in range(KT):
                    nc.tensor.matmul(ps[:, 0:BL],
                                     w2t_sb[ki][:, mo * 128:(mo + 1) * 128],
                                     te1_sb[ki][:, :],
                                     start=(ki == 0), stop=(ki == KT - 1))
                nc.scalar.activation(te_sb[mo][:, :], ps[:, 0:BL], AF.Relu,
                                     bias=bt2_sb[mo][:, :])

            # ================= BN1-ko0 -> conv2 ki0 phase ==================
            scl0, shf0 = bn_consts(0, 0, 0)
            for s in range(BL):
                bsk = pst.tile([128, 1], F32, name=f"b1s{s}0")
                nc.vector.tensor_tensor(bsk[:, :], shf0[:, :],
                                        te_sb[0][:, s:s + 1], ALU.add)
                nc.scalar.activation(_interior(h1_pad[s][0]),
                                     _interior(h1_pad[s][0]), AF.Identity,
                                     bias=bsk[:, :], scale=scl0[:, :])

            filler(fill["A"])
            # 7 psums span the phase boundary; the 8th (s1,ko1,h1) runs as a
            # plain 18-chain in the ki1 phase.
            c2keys = [(s, ko, half) for s in range(BL) for ko in range(KT)
                      for half in range(2)]
            c2open = c2keys[:-1]
            c2late = c2keys[-1]
            c2ps = {}
            for (s, ko, half) in c2open:
                ps = psum(f"c2_{s}{ko}{half}")
                c2ps[(s, ko, half)] = ps
                ps3 = ps[:, :].rearrange("p (r c) -> p r c", c=W)
                conv_part(1, h1_pad, s, ko, half, 0, ps3, first=True, last=False)

            # ---- BN1-ko1 -> normalize -> conv2 ki1 (ko-ordered) ----
            scl1, shf1 = bn_consts(1, 0, 1)
            for s in range(BL):
                bsk = pst.tile([128, 1], F32, name=f"b1s{s}1")
                nc.vector.tensor_tensor(bsk[:, :], shf1[:, :],
                                        te_sb[1][:, s:s + 1], ALU.add)
                nc.scalar.activation(_interior(h1_pad[s][1]),
                                     _interior(h1_pad[s][1]), AF.Identity,
                                     bias=bsk[:, :], scale=scl1[:, :])

            h2_pad = [[ppad.tile([128, NPAD], dt_conv, tag="pad", name=f"h2p{s}{k}")
                       for k in range(KT)] for s in range(BL)]
            for s in range(BL):
                for k in range(KT):
                    _memset_border(nc, h2_pad[s][k])

            filler(fill["B"])
            for ko in range(KT):
                for (s, ko_, half) in c2keys:
                    if ko_ != ko:
                        continue
                    if (s, ko_, half) == c2late:
                        ps = psum(f"c2_{s}{ko_}{half}")
                        ps3 = ps[:, :].rearrange("p (r c) -> p r c", c=W)
                        for ki in range(KT):
                            conv_part(1, h1_pad, s, ko_, half, ki, ps3,
                                      first=(ki == 0), last=(ki == KT - 1))
                    else:
                        ps = c2ps[(s, ko_, half)]
                        ps3 = ps[:, :].rearrange("p (r c) -> p r c", c=W)
                        conv_part(1, h1_pad, s, ko_, half, 1, ps3,
                                  first=False, last=True)
                    epilogue_stats(1, h2_pad, s, ko_, half, ps3)
                chunk_sync(1, ko)

            # ================= BN2-ko0 -> conv3 ki0 phase ==================
            scl2, shf2 = bn_consts(2, 1, 0)
            for s in range(BL):
                nc.scalar.activation(_interior(h2_pad[s][0]),
                                     _interior(h2_pad[s][0]), AF.Identity,
                                     bias=shf2[:, :], scale=scl2[:, :])

            filler(fill["C"])
            c3open = c2keys[:-1]
            c3late = c2keys[-1]
            c3ps = {}
            for (s, ko, half) in c3open:
                ps = psum(f"c3_{s}{ko}{half}")
                c3ps[(s, ko, half)] = ps
                ps3 = ps[:, :].rearrange("p (r c) -> p r c", c=W)
                conv_part(2, h2_pad, s, ko, half, 0, ps3, first=True, last=False)

            scl3, shf3 = bn_consts(3, 1, 1)
            for s in range(BL):
                nc.scalar.activation(_interior(h2_pad[s][1]),
                                     _interior(h2_pad[s][1]), AF.Identity,
                                     bias=shf3[:, :], scale=scl3[:, :])

            y_sb = [[py.tile([128, N], dt_attn, tag="y", name=f"y{s}{k}")
                     for k in range(KT)] for s in range(BL)]
            filler(fill["D"])
            for (s, ko, half) in c2keys:
                if (s, ko, half) == c3late:
                    ps = psum(f"c3_{s}{ko}{half}")
                    ps3 = ps[:, :].rearrange("p (r c) -> p r c", c=W)
                    for ki in range(KT):
                        conv_part(2, h2_pad, s, ko, half, ki, ps3,
                                  first=(ki == 0), last=(ki == KT - 1))
                else:
                    ps = c3ps[(s, ko, half)]
                    ps3 = ps[:, :].rearrange("p (r c) -> p r c", c=W)
                    conv_part(2, h2_pad, s, ko, half, 1, ps3,
                              first=False, last=True)
                nc.scalar.activation(
                    y_sb[s][ko][:, half * 512:(half + 1) * 512],
                    ps[:, :], AF.Identity, bias=cb_sb[2][ko][:, :])

            # ================= attention (per sample) ======================
            filler(fill["E"])
            for s in range(BL):
                vt = []
                filler(fill["V"])
                for nt in range(8):
                    ps = psum(f"vps{s}{nt}")
                    pv = ps[:, 0:C]
                    for c2 in range(KT):
                        nc.tensor.matmul(pv, y_sb[s][c2][:, nt * 128:(nt + 1) * 128],
                                         wvt_sb[c2][:, :], start=(c2 == 0), stop=False)
                    nc.tensor.matmul(pv, ones_row[:, :], bv_sb[:, :],
                                     start=False, stop=True)
                    v = pat.tile([128, C], dt_attn, tag="vt", bufs=9, name=f"vt{s}{nt}")
                    nc.vector.tensor_copy(v[:, :], pv)
                    vt.append(v)

                q_sb = pat.tile([CQ, N], dt_attn, tag="q", bufs=2, name=f"q{s}")
                k_sb = pat.tile([CQ, N], dt_attn, tag="k", bufs=2, name=f"k{s}")
                for nh in range(2):
                    psq_ = psum(f"qps{s}{nh}")
                    for c2 in range(KT):
                        nc.tensor.matmul(psq_[0:CQ, :], wqt_sb[c2][:, :],
                                         y_sb[s][c2][:, nh * 512:(nh + 1) * 512],
                                         start=(c2 == 0), stop=(c2 == KT - 1))
                    nc.scalar.activation(q_sb[:, nh * 512:(nh + 1) * 512],
                                         psq_[0:CQ, :], AF.Identity, bias=bq_sb[:, :])
                    psk_ = psum(f"kps{s}{nh}")
                    for c2 in range(KT):
                        nc.tensor.matmul(psk_[0:CQ, :], wkt_sb[c2][:, :],
                                         y_sb[s][c2][:, nh * 512:(nh + 1) * 512],
                                         start=(c2 == 0), stop=(c2 == KT - 1))
                    nc.scalar.activation(k_sb[:, nh * 512:(nh + 1) * 512],
                                         psk_[0:CQ, :], AF.Identity, bias=bk_sb[:, :])

                res_t = [pat.tile([128, N], F32R, tag="res", bufs=4,
                                  name=f"res{s}{c2}") for c2 in range(KT)]
                for nh in range(2):
                    filler(fill["S"])
                    ptiles = []
                    for mt in range(8):
                        ps = psum(f"sps{s}{nh}{mt}")
                        nc.tensor.matmul(ps[:, :], k_sb[:, mt * 128:(mt + 1) * 128],
                                         q_sb[:, nh * 512:(nh + 1) * 512],
                                         start=True, stop=True)
                        p = pat.tile([128, 512], dt_attn, tag="P", bufs=16,
                                     name=f"P{s}{nh}{mt}")
                        nc.scalar.activation(p[:, :], ps[:, :], AF.Exp)
                        ptiles.append(p)
                    # V @ P output matmuls first in PE order
                    pr_ps = []
                    for c2 in range(KT):
                        pr = psum(f"rps{s}{nh}{c2}")
                        pr_ps.append(pr)
                        for mt in range(8):
                            nc.tensor.matmul(pr[:, :],
                                             vt[mt][:, c2 * 128:(c2 + 1) * 128],
                                             ptiles[mt][:, :],
                                             start=(mt == 0), stop=(mt == 7))
                    # denominator: two DVE add-trees + 2 accumulating
                    # ones-matmuls (short tail)
                    pacc = [pat.tile([128, 512], dt_attn, tag="pacc", bufs=2,
                                     name=f"pacc{s}{nh}{h}") for h in range(2)]
                    for h in range(2):
                        nc.vector.tensor_tensor(pacc[h][:, :],
                                                ptiles[4 * h][:, :],
                                                ptiles[4 * h + 1][:, :], ALU.add)
                        nc.vector.tensor_tensor(pacc[h][:, :], pacc[h][:, :],
                                                ptiles[4 * h + 2][:, :], ALU.add)
                        nc.vector.tensor_tensor(pacc[h][:, :], pacc[h][:, :],
                                                ptiles[4 * h + 3][:, :], ALU.add)
                    pd = psum(f"dps{s}{nh}")
                    nc.tensor.matmul(pd[0:1, :], ones_col[:, :], pacc[0][:, :],
                                     start=True, stop=False)
                    nc.tensor.matmul(pd[0:1, :], ones_col[:, :], pacc[1][:, :],
                                     start=False, stop=True)
                    rcp = pat.tile([1, 512], dt_attn, tag="rcp", bufs=2,
                                   name=f"rcp{s}{nh}")
                    with nc.allow_low_precision(reason="f32r==f32 bit layout"):
                        nc.vector.reciprocal(rcp[:, :], pd[0:1, :])
                    pb = psum(f"bps{s}{nh}")
                    nc.tensor.matmul(pb[:, :], ones_row[:, :], rcp[:, :],
                                     start=True, stop=True)
                    rb = pat.tile([128, 512], F32, tag="rb", bufs=2, name=f"rb{s}{nh}")
                    nc.vector.tensor_copy(rb[:, :], pb[:, :])
                    # out = (V @ P) * rb + y  (gamma folded into wv/bv on host)
                    for c2 in range(KT):
                        rs = res_t[c2][:, nh * 512:(nh + 1) * 512]
                        nc.vector.tensor_tensor(rs, pr_ps[c2][:, :], rb[:, :],
                                                ALU.mult)
                        nc.vector.tensor_tensor(rs, rs,
                                                y_sb[s][c2][:, nh * 512:(nh + 1) * 512],
                                                ALU.add)
                        nc.sync.dma_start(
                            out=out_d[s, c2, :, nh * 512:(nh + 1) * 512],
                            in_=rs)

    if split:
        _split_packed_waits(nc)
    return nc


def _prep_inputs(inputs):
    f32 = np.float32
    x = np.asarray(inputs["x"], f32)
    t = np.asarray(inputs["t"], f32)

    def conv_w(w):
        w6 = np.asarray(w, f32).reshape(KT, 128, KT, 128, 3, 3)  # ko,o,ki,i,dy,dx
        arr = w6.transpose(3, 0, 4, 5, 2, 1)  # i,ko,dy,dx,ki,o
        return np.ascontiguousarray(arr.reshape(128, KT * KCOLS))

    cw = np.stack([conv_w(inputs["w_c1"]), conv_w(inputs["w_c2"]),
                   conv_w(inputs["w_tr"])])
    w1t = np.ascontiguousarray(np.asarray(inputs["w_t1"], f32).T.reshape(KT, 128, T))
    w2t = np.ascontiguousarray(np.asarray(inputs["w_t2"], f32).T.reshape(KT, 128, C))
    consts = np.zeros((128, 22), f32)
    for ci, k2 in enumerate(("b_c1", "b_c2", "b_tr")):
        consts[:, ci * KT:(ci + 1) * KT] = np.asarray(inputs[k2], f32).reshape(KT, 128).T
    for i, (gk, bk2) in enumerate((("bn1_g", "bn1_b"), ("bn2_g", "bn2_b"))):
        consts[:, 6 + i * KT:6 + (i + 1) * KT] = np.asarray(inputs[gk], f32).reshape(KT, 128).T
        consts[:, 10 + i * KT:10 + (i + 1) * KT] = np.asarray(inputs[bk2], f32).reshape(KT, 128).T
    consts[:, 14:16] = np.asarray(inputs["b_t1"], f32).reshape(KT, 128).T
    consts[:, 16:18] = np.asarray(inputs["b_t2"], f32).reshape(KT, 128).T
    gam = float(np.asarray(inputs["gamma"], f32).reshape(()))
    wqt = np.ascontiguousarray(np.asarray(inputs["wq"], f32).T.reshape(KT, 128, CQ))
    wkt = np.ascontiguousarray(np.asarray(inputs["wk"], f32).T.reshape(KT, 128, CQ))
    # gamma folded into V projection (out = gamma*out_att + y)
    wvt = np.ascontiguousarray(
        (gam * np.asarray(inputs["wv"], f32)).T.reshape(KT, 128, C))
    bq = np.asarray(inputs["bq"], f32).reshape(CQ, 1)
    bk = np.asarray(inputs["bk"], f32).reshape(CQ, 1)
    bv = (gam * np.asarray(inputs["bv"], f32)).reshape(1, C)

    xp = np.zeros((B, KT, 128, HP, WP), f32)
    xp[:, :, :, 1:1 + H, 1:1 + W] = x.reshape(B, KT, 128, H, W)
    xp = xp.reshape(B, KT, 128, NPAD)
    ttr = np.ascontiguousarray(t.T.reshape(KT, 128, B))

    shared = dict(cw=cw, w1t=w1t, w2t=w2t,
                  wqt=wqt, wkt=wkt, wvt=wvt, bq=bq, bk=bk, bv=bv)
    per_core = []
    for c in range(NCORES):
        m = dict(shared)
        m["xp"] = np.ascontiguousarray(xp[c * BL:(c + 1) * BL])
        cc_consts = consts.copy()
        for k in range(KT):
            cc_consts[:, 18 + k * BL:18 + (k + 1) * BL] = \
                ttr[k, :, c * BL:(c + 1) * BL]
        m["consts"] = cc_consts
        per_core.append(m)
    return per_core


def _unshard(results):
    out = np.empty((B, C, H, W), np.float32)
    for c in range(NCORES):
        o = results[c]["out"].reshape(BL, KT, 128, H, W)
        for s in range(BL):
            out[c * BL + s] = o[s].reshape(C, H, W)
    return out


_cache = {}

DT_CONV = F32R
DT_ATTN = F32R


def kernel(**inputs) -> np.ndarray:
    key = ("nc", str(DT_CONV), str(DT_ATTN))
    if key not in _cache:
        _cache[key] = build(dt_conv=DT_CONV, dt_attn=DT_ATTN)
    nc = _cache[key]
    per_core = _prep_inputs(inputs)
    try:
        res = run_bass_kernel_spmd(nc, per_core, core_ids=list(range(NCORES)))
    except Exception:
        # transient NRT_EXEC_UNIT_UNRECOVERABLE errors recover on re-execute
        res = run_bass_kernel_spmd(nc, per_core, core_ids=list(range(NCORES)))
    return _unshard(res.results)


# revision 14
# speedup vs baseline: 1.3924x; 1.0612x over previous
"""Trainium2 Bass kernel for nn_BlockWithAttention (dense CNN block + attention).

Sharding: data-parallel over batch (B=16 -> 2 samples/core x 8 cores).

Scheduling design (tuned against the TimelineSim cost model):
- The PE pstate ramp model freezes each matmul's clock at cost time, so any
  PE idle gap poisons the following flood with 2-3.7x slower matmuls. Tiny
  "filler" matmuls ([128,32] ones, dedicated PSUM bank) bridge every
  potential idle window so real matmuls always cost at the 2.4GHz peak.
- BN batch-stat sync is chunked per 128-channel group (2 AllGathers per BN)
  and pipelined: BN1-ko0's collective flies during conv1-ko1, BN1-ko1's
  during conv2's ki0 phase, BN2's two during conv2-ki1/conv3-ki0.
- conv2/conv3 split into ki phases with 7 psums held open across the phase
  boundary (the 8th PSUM bank belongs to the fillers); the 8th conv psum
  runs as a plain 18-chain in the ki1 phase.
- attention: V@P output matmuls are emitted before the softmax-denominator
  reduce; the denominator add-tree is split in two for a shorter tail;
  gamma is folded into wv/bv host-side.

All matmuls run in float32r (full PE rate at free-size>=256); accumulation
is fp32 in PSUM.
"""
import numpy as np

import concourse.bass as bass
import concourse.mybir as mybir
from concourse.bass_utils import run_bass_kernel_spmd
from concourse.tile import TileContext
from concourse.tile_rust import add_dep_helper

# ---- problem constants ----
B, C, H, W, T, CQ = 16, 256, 32, 32, 256, 32
NCORES = 8
BL = B // NCORES            # samples per core
KT = C // 128               # 128-channel tiles
HP, WP = H + 2, W + 2       # padded image
NPAD = HP * WP              # 1156
NPIX = B * H * W            # BN stat count (full batch)
N = H * W                   # 1024 spatial positions
RH = 16                     # rows per 512-px half
EPS = 1e-5
KCOLS = 9 * KT * 128        # weight cols per ko group (ko-major layout)

F32 = mybir.dt.float32
F32R = mybir.dt.float32r
BF16 = mybir.dt.bfloat16
AX = mybir.AxisListType
ALU = mybir.AluOpType
AF = mybir.ActivationFunctionType

U32 = mybir.dt.uint32
U16 = mybir.dt.uint16
ONE_F32_BITS = 0x3F800000

# conv1/conv2 operand dtype: bf16 halves DMA; set False to use f32r
USE_BF16 = False
CDT = BF16 if USE_BF16 else F32R

# filler-block sizes (tuned against TimelineSim)
FILL = {
    "ign0": 130,   # kernel start -> first conv1 matmul
    "A": 60,       # conv1/MLP -> conv2-ki0 (BN1-ko0 wait)
    "B": 15,       # conv2-ki0 -> conv2-ki1 (BN1-ko1 wait, insurance)
    "C": 240,      # conv2 end -> conv3-ki0 (BN2-ko0 collective wait)
    "D": 40,       # conv3-ki0 -> conv3-ki1 (insurance)
    "E": 25,       # conv3 -> attention V (y epilogue trail)
    "V": 20,       # per-sample V block lead-in
    "S": 20,       # per-(s,nh) S block lead-in
}

_wsplit_counter = [0]


def _split_packed_waits(nc, max_waits: int = 1):
    """The walrus build here rejects >1-2 packed sync-waits per instruction
    ("Too many sync wait commands"). Move excess waits onto standalone
    single-wait EventSemaphore carriers inserted before the instruction
    (same engine -> program order preserves gating)."""
    for f in nc.m.functions:
        for bb in f.blocks:
            il = bb.instructions
            i = 0
            while i < len(il):
                inst = il[i]
                si = inst.sync_info
                if si is not None and len(si.on_wait) > max_waits:
                    waits = list(si.on_wait)
                    movable = [w for w in waits if w.wait_reg is None]
                    fixed = [w for w in waits if w.wait_reg is not None]
                    keep_n = max(0, max_waits - len(fixed))
                    kept = fixed + movable[:keep_n]
                    move = movable[keep_n:]
                    if not move:
                        i += 1
                        continue
                    si.on_wait = kept
                    for w in move:
                        _wsplit_counter[0] += 1
                        ev = mybir.InstEventSemaphore(
                            name=f"I-wsplit-{_wsplit_counter[0]}",
                            opcode="EventSemaphore",
                            engine=inst.engine,
                            sync_info=mybir.SyncInfo(on_wait=[w], on_update=[]),
                        )
                        il.insert(i, ev)
                        i += 1
                i += 1


def _pad3(tile):
    return tile[:, :].rearrange("p (r c) -> p r c", c=WP)


def _interior(tile, r0=0, nr=H):
    return _pad3(tile)[:, 1 + r0:1 + r0 + nr, 1:1 + W]


def _tap(tile, dy, dx, r0, nr):
    return _pad3(tile)[:, r0 + dy:r0 + dy + nr, dx:dx + W]


def _memset_border(nc, tile):
    # gpsimd memset rejects float32r; write zero bits via an int bitcast of
    # matching width
    cast = U32 if mybir.dt.size(tile.dtype) == 4 else U16
    v = _pad3(tile)
    nc.gpsimd.memset(v[:, 0:1, :].bitcast(cast), 0)
    nc.gpsimd.memset(v[:, HP - 1:HP, :].bitcast(cast), 0)
    nc.gpsimd.memset(v[:, 1:HP - 1, 0:1].bitcast(cast), 0)
    nc.gpsimd.memset(v[:, 1:HP - 1, WP - 1:WP].bitcast(cast), 0)


def build(dt_conv=F32R, dt_attn=F32R, split: bool = True, fill=None):
    fill = dict(FILL, **(fill or {}))
    nc = bass.Bass(num_devices=NCORES)

    # ---- DRAM I/O ----
    xp_d = nc.dram_tensor("xp", [BL, KT, 128, NPAD], CDT, kind="ExternalInput")
    # ko-major conv weights: [ci][128(i), ((ko*9 + tap)*KT + ki)*128 + o]
    # conv1/conv2 in bf16 (errors laundered by the BNs); conv3 stays f32r so
    # the attention logits (exp-amplified) keep full precision.
    cwa_d = nc.dram_tensor("cwa", [2, 128, KT * KCOLS], CDT, kind="ExternalInput")
    cw2_d = nc.dram_tensor("cw2", [128, KT * KCOLS], F32R, kind="ExternalInput")
    w1t_d = nc.dram_tensor("w1t", [KT, 128, T], F32R, kind="ExternalInput")
    w2t_d = nc.dram_tensor("w2t", [KT, 128, C], F32R, kind="ExternalInput")
    consts_d = nc.dram_tensor("consts", [128, 22], F32R, kind="ExternalInput")
    wqt_d = nc.dram_tensor("wqt", [KT, 128, CQ], dt_attn, kind="ExternalInput")
    wkt_d = nc.dram_tensor("wkt", [KT, 128, CQ], dt_attn, kind="ExternalInput")
    wvt_d = nc.dram_tensor("wvt", [KT, 128, C], dt_attn, kind="ExternalInput")
    bq_d = nc.dram_tensor("bq", [CQ, 1], F32R, kind="ExternalInput")
    bk_d = nc.dram_tensor("bk", [CQ, 1], F32R, kind="ExternalInput")
    bv_d = nc.dram_tensor("bv", [1, C], dt_attn, kind="ExternalInput")
    out_d = nc.dram_tensor("out", [BL, KT, 128, N], F32R, kind="ExternalOutput")

    cc_in = [nc.dram_tensor(f"cc{i}_in", [128, 2, 4], F32) for i in range(4)]
    cc_out = [nc.dram_tensor(f"cc{i}_out", [NCORES, 128, 8], F32,
                             addr_space="Shared") for i in range(4)]

    with TileContext(nc) as tc:
        with (
            tc.tile_pool(name="pconst", bufs=1) as pc,
            tc.tile_pool(name="pcw", bufs=2) as pcw,
            tc.tile_pool(name="ppad", bufs=8) as ppad,
            tc.tile_pool(name="py", bufs=4) as py,
            tc.tile_pool(name="psq", bufs=2) as psq,
            tc.tile_pool(name="pattn", bufs=1) as pat,
            tc.tile_pool(name="pstats", bufs=1) as pst,
            tc.tile_pool(name="ppsum", bufs=1, space="PSUM") as pps,
        ):
            def psum(nm):
                return pps.tile([128, 512], F32, tag="ps", bufs=7, name=nm)

            # ---- filler infrastructure: dedicated PSUM bank + ones tile ----
            fones = pc.tile([128, 32], F32R, name="fones")
            nc.gpsimd.memset(fones[:, :].bitcast(U32), ONE_F32_BITS)
            fps = pps.tile([32, 512], F32, tag="fill", bufs=1, name="fps")

            def filler(n):
                for _ in range(n):
                    nc.tensor.matmul(fps[0:32, 0:32], fones[:, :], fones[:, :],
                                     start=True, stop=True)

            # ---- big DMAs: x first, conv1 weights in ko-major halves ----
            cw_sb = [pcw.tile([128, KT * KCOLS], CDT if ci < 2 else F32R,
                              tag="cw", bufs=2, name=f"cw{ci}")
                     for ci in range(3)]
            x_pad = [[ppad.tile([128, NPAD], CDT, tag="xpad", bufs=4,
                                name=f"xp{s}{k}")
                      for k in range(KT)] for s in range(BL)]
            for k in range(KT):
                nc.sync.dma_start(out=x_pad[0][k][:, :], in_=xp_d[0, k, :, :])
            nc.sync.dma_start(out=cw_sb[0][:, 0:KCOLS], in_=cwa_d[0, :, 0:KCOLS])
            for k in range(KT):
                nc.sync.dma_start(out=x_pad[1][k][:, :], in_=xp_d[1, k, :, :])
            nc.sync.dma_start(out=cw_sb[0][:, KCOLS:2 * KCOLS],
                              in_=cwa_d[0, :, KCOLS:2 * KCOLS])

            # ---- persistent small tiles ----
            w1t_sb = [pc.tile([128, T], F32R, name=f"w1t{k}") for k in range(KT)]
            w2t_sb = [pc.tile([128, C], F32R, name=f"w2t{k}") for k in range(KT)]
            consts_sb = pc.tile([128, 22], F32R, name="consts_sb")

            def ccol(j, n=1):
                return consts_sb[:, j:j + n]

            cb_sb = [[ccol(ci * KT + k) for k in range(KT)] for ci in range(3)]
            bng_sb = [[ccol(6 + i * KT + k) for k in range(KT)] for i in range(2)]
            bnb_sb = [[ccol(10 + i * KT + k) for k in range(KT)] for i in range(2)]
            bt1_sb = [ccol(14 + k) for k in range(KT)]
            bt2_sb = [ccol(16 + k) for k in range(KT)]
            tt_sb = [ccol(18 + k * BL, BL) for k in range(KT)]
            wqt_sb = [pc.tile([128, CQ], dt_attn, name=f"wqt{k}") for k in range(KT)]
            wkt_sb = [pc.tile([128, CQ], dt_attn, name=f"wkt{k}") for k in range(KT)]
            wvt_sb = [pc.tile([128, C], dt_attn, name=f"wvt{k}") for k in range(KT)]
            bq_sb = pc.tile([CQ, 1], F32R, name="bq_sb")
            bk_sb = pc.tile([CQ, 1], F32R, name="bk_sb")
            bv_sb = pc.tile([1, C], dt_attn, name="bv_sb")
            ones_col = pc.tile([128, 1], dt_attn, name="ones_col")
            ones_row = pc.tile([1, 128], dt_attn, name="ones_row")

            nc.gpsimd.dma_start(out=consts_sb[:, :], in_=consts_d[:, :])
            for k in range(KT):
                nc.sync.dma_start(out=w1t_sb[k][:, :], in_=w1t_d[k, :, :])
                nc.sync.dma_start(out=w2t_sb[k][:, :], in_=w2t_d[k, :, :])
                nc.sync.dma_start(out=wqt_sb[k][:, :], in_=wqt_d[k, :, :])
                nc.sync.dma_start(out=wkt_sb[k][:, :], in_=wkt_d[k, :, :])
                nc.sync.dma_start(out=wvt_sb[k][:, :], in_=wvt_d[k, :, :])
            nc.gpsimd.dma_start(out=bq_sb[:, :], in_=bq_d[:, :])
            nc.gpsimd.dma_start(out=bk_sb[:, :], in_=bk_d[:, :])
            nc.gpsimd.dma_start(out=bv_sb[:, :], in_=bv_d[:, :])
            nc.gpsimd.memset(ones_col[:, :].bitcast(U32), ONE_F32_BITS)
            nc.gpsimd.memset(ones_row[:, :].bitcast(U32), ONE_F32_BITS)
            nc.sync.dma_start(out=cw_sb[1][:, :], in_=cwa_d[1, :, :])
            nc.sync.dma_start(out=cw_sb[2][:, 0:KCOLS], in_=cw2_d[:, 0:KCOLS])
            nc.sync.dma_start(out=cw_sb[2][:, KCOLS:2 * KCOLS],
                              in_=cw2_d[:, KCOLS:2 * KCOLS])

            stats = [pst.tile([128, 16], F32, name=f"stats{i}") for i in range(2)]
            glob = [pst.tile([128, 2], F32, name=f"glob{i}") for i in range(4)]
            gall = [pst.tile([128, 8 * NCORES], F32, name=f"gall{i}")
                    for i in range(4)]
            for i in range(2):
                nc.gpsimd.memset(stats[i][:, :], 0.0)

            h1_pad = [[ppad.tile([128, NPAD], CDT, tag="h1pad", bufs=4,
                                 name=f"h1p{s}{k}")
                       for k in range(KT)] for s in range(BL)]
            for s in range(BL):
                for k in range(KT):
                    _memset_border(nc, h1_pad[s][k])

            # ---- helpers ----
            def conv_part(ci, src_pads, s, ko, half, ki, ps3, first, last):
                r0 = half * RH
                for tap in range(9):
                    dy, dx = divmod(tap, 3)
                    j = (ko * 9 + tap) * KT + ki
                    nc.tensor.matmul(
                        ps3,
                        cw_sb[ci][:, j * 128:(j + 1) * 128],
                        _tap(src_pads[s][ki], dy, dx, r0, RH),
                        start=(first and tap == 0), stop=(last and tap == 8),
                    )

            def epilogue_stats(i, dst_pad, s, ko, half, ps3):
                col = ko * 4 + s * 2 + half
                nc.scalar.activation(
                    _interior(dst_pad[s][ko], half * RH, RH), ps3, AF.Relu,
                    bias=cb_sb[i][ko][:, :],
                    accum_out=stats[i][:, col:col + 1],
                )
                sq = psq.tile([128, 512], F32, tag="sq", name=f"sq{i}_{s}{ko}{half}")
                nc.scalar.activation(
                    sq[:, :].rearrange("p (r c) -> p r c", c=W),
                    _interior(dst_pad[s][ko], half * RH, RH), AF.Square,
                    accum_out=stats[i][:, 8 + col:9 + col],
                )

            def chunk_sync(i, ko):
                ci_ = i * KT + ko
                # ship raw (s,half) accumulator columns; one reduce after
                # the gather covers both the core and (s,half) axes
                d1 = nc.sync.dma_start(
                    out=cc_in[ci_][:, :],
                    in_=stats[i][:, :].rearrange("p (k c) -> p k c", k=2)
                        [:, :, ko * 4:ko * 4 + 4])
                cc = nc.gpsimd.collective_compute(
                    "AllGather", ALU.bypass,
                    replica_groups=[list(range(NCORES))],
                    ins=[cc_in[ci_][:].opt()], outs=[cc_out[ci_][:].opt()],
                )
                add_dep_helper(cc.ins, d1.ins, reason="cc waits on stats dma")
                d2 = nc.sync.dma_start(
                    out=gall[ci_][:, :],
                    in_=cc_out[ci_][:, :, :].rearrange("c p k -> p c k"))
                add_dep_helper(d2.ins, cc.ins, reason="readback waits on cc")
                gk = pst.tile([128, 2 * NCORES], F32, name=f"gk{ci_}")
                nc.vector.reduce_sum(
                    gk[:, :],
                    gall[ci_][:, :].rearrange("p (ck s) -> p ck s", s=4),
                    axis=AX.X)
                nc.vector.reduce_sum(
                    glob[ci_][:, :],
                    gk[:, :].rearrange("p (c k) -> p k c", k=2), axis=AX.X)

            def bn_consts(ci_, i, ko):
                mean = pst.tile([128, 1], F32, name=f"mean{ci_}")
                ex2 = pst.tile([128, 1], F32, name=f"ex2{ci_}")
                var = pst.tile([128, 1], F32, name=f"var{ci_}")
                rv = pst.tile([128, 1], F32, name=f"rv{ci_}")
                scl = pst.tile([128, 1], F32, name=f"scl{ci_}")
                shf = pst.tile([128, 1], F32, name=f"shf{ci_}")
                nc.vector.tensor_scalar_mul(mean[:, :], glob[ci_][:, 0:1], 1.0 / NPIX)
                nc.vector.tensor_scalar_mul(ex2[:, :], glob[ci_][:, 1:2], 1.0 / NPIX)
                nc.vector.tensor_tensor(var[:, :], mean[:, :], mean[:, :], ALU.mult)
                nc.vector.tensor_tensor(var[:, :], ex2[:, :], var[:, :], ALU.subtract)
                nc.vector.tensor_scalar(out=var[:, :], in0=var[:, :], scalar1=EPS,
                                        scalar2=None, op0=ALU.add)
                nc.vector.reciprocal(rv[:, :], var[:, :])
                nc.scalar.activation(rv[:, :], rv[:, :], AF.Sqrt)
                # Newton step: y' = 0.5*y*(3 - v*y^2)
                t1 = pst.tile([128, 1], F32, name=f"nr{ci_}")
                nc.vector.tensor_tensor(t1[:, :], rv[:, :], rv[:, :], ALU.mult)
                nc.vector.tensor_tensor(t1[:, :], var[:, :], t1[:, :], ALU.mult)
                nc.vector.tensor_scalar(out=t1[:, :], in0=t1[:, :], scalar1=-1.0,
                                        scalar2=3.0, op0=ALU.mult, op1=ALU.add)
                nc.vector.tensor_tensor(t1[:, :], rv[:, :], t1[:, :], ALU.mult)
                nc.vector.tensor_scalar_mul(rv[:, :], t1[:, :], 0.5)
                nc.vector.tensor_tensor(scl[:, :], rv[:, :], bng_sb[i][ko][:, :],
                                        ALU.mult)
                nc.vector.tensor_tensor(shf[:, :], mean[:, :], scl[:, :], ALU.mult)
                nc.vector.tensor_tensor(shf[:, :], bnb_sb[i][ko][:, :], shf[:, :],
                                        ALU.subtract)
                return scl, shf

            # ================= ignition + conv1 (chunked by ko) ============
            filler(fill["ign0"])
            for ko in range(KT):
                for s in range(BL):
                    for half in range(2):
                        ps = psum(f"c1_{s}{ko}{half}")
                        ps3 = ps[:, :].rearrange("p (r c) -> p r c", c=W)
                        for ki in range(KT):
                            conv_part(0, x_pad, s, ko, half, ki, ps3,
                                      first=(ki == 0), last=(ki == KT - 1))
                        epilogue_stats(0, h1_pad, s, ko, half, ps3)
                chunk_sync(0, ko)

            # ---- time MLP ----
            te1_sb = [pst.tile([128, BL], F32R, name=f"te1_{m}") for m in range(KT)]
            te_sb = [pst.tile([128, BL], F32R, name=f"te_{m}") for m in range(KT)]
            for mo in range(KT):
                ps = psum(f"mlp1_{mo}")
                for ki in range(KT):
                    nc.tensor.matmul(ps[:, 0:BL],
                                     w1t_sb[ki][:, mo * 128:(mo + 1) * 128],
                                     tt_sb[ki][:, :],
                                     start=(ki == 0), stop=(ki == KT - 1))
                nc.scalar.activation(te1_sb[mo][:, :], ps[:, 0:BL], AF.Relu,
                                     bias=bt1_sb[mo][:, :])
            for mo in range(KT):
                ps = psum(f"mlp2_{mo}")
                for ki in range(KT):
                    nc.tensor.matmul(ps[:, 0:BL],
                                     w2t_sb[ki][:, mo * 128:(mo + 1) * 128],
                                     te1_sb[ki][:, :],
                                     start=(ki == 0), stop=(ki == KT - 1))
                nc.scalar.activation(te_sb[mo][:, :], ps[:, 0:BL], AF.Relu,
                                     bias=bt2_sb[mo][:, :])

            # ================= BN1-ko0 -> conv2 ki0 phase ==================
            scl0, shf0 = bn_consts(0, 0, 0)
            for s in range(BL):
                bsk = pst.tile([128, 1], F32, name=f"b1s{s}0")
                nc.vector.tensor_tensor(bsk[:, :], shf0[:, :],
                                        te_sb[0][:, s:s + 1], ALU.add)
                nc.vector.tensor_scalar(out=_interior(h1_pad[s][0]),
                                        in0=_interior(h1_pad[s][0]),
                                        scalar1=scl0[:, :], scalar2=bsk[:, :],
                                        op0=ALU.mult, op1=ALU.add)

            filler(fill["A"])
            # 7 psums span the phase boundary; the 8th (s1,ko1,h1) runs as a
            # plain 18-chain in the ki1 phase.
            c2keys = [(s, ko, half) for s in range(BL) for ko in range(KT)
                      for half in range(2)]
            c2open = c2keys[:-1]
            c2late = c2keys[-1]
            c2ps = {}
            for (s, ko, half) in c2open:
                ps = psum(f"c2_{s}{ko}{half}")
                c2ps[(s, ko, half)] = ps
                ps3 = ps[:, :].rearrange("p (r c) -> p r c", c=W)
                conv_part(1, h1_pad, s, ko, half, 0, ps3, first=True, last=False)

            # ---- BN1-ko1 -> normalize -> conv2 ki1 (ko-ordered) ----
            scl1, shf1 = bn_consts(1, 0, 1)
            for s in range(BL):
                bsk = pst.tile([128, 1], F32, name=f"b1s{s}1")
                nc.vector.tensor_tensor(bsk[:, :], shf1[:, :],
                                        te_sb[1][:, s:s + 1], ALU.add)
                nc.vector.tensor_scalar(out=_interior(h1_pad[s][1]),
                                        in0=_interior(h1_pad[s][1]),
                                        scalar1=scl1[:, :], scalar2=bsk[:, :],
                                        op0=ALU.mult, op1=ALU.add)

            h2_pad = [[ppad.tile([128, NPAD], F32R, tag="h2pad", bufs=4,
                                 name=f"h2p{s}{k}")
                       for k in range(KT)] for s in range(BL)]
            for s in range(BL):
                for k in range(KT):
                    _memset_border(nc, h2_pad[s][k])

            filler(fill["B"])
            for ko in range(KT):
                for (s, ko_, half) in c2keys:
                    if ko_ != ko:
                        continue
                    if (s, ko_, half) == c2late:
                        ps = psum(f"c2_{s}{ko_}{half}")
                        ps3 = ps[:, :].rearrange("p (r c) -> p r c", c=W)
                        for ki in range(KT):
                            conv_part(1, h1_pad, s, ko_, half, ki, ps3,
                                      first=(ki == 0), last=(ki == KT - 1))
                    else:
                        ps = c2ps[(s, ko_, half)]
                        ps3 = ps[:, :].rearrange("p (r c) -> p r c", c=W)
                        conv_part(1, h1_pad, s, ko_, half, 1, ps3,
                                  first=False, last=True)
                    epilogue_stats(1, h2_pad, s, ko_, half, ps3)
                chunk_sync(1, ko)

            # ================= BN2-ko0 -> conv3 ki0 phase ==================
            scl2, shf2 = bn_consts(2, 1, 0)
            with nc.allow_low_precision(reason="f32r==f32 bit layout"):
                for s in range(BL):
                    nc.vector.tensor_scalar(out=_interior(h2_pad[s][0]),
                                            in0=_interior(h2_pad[s][0]),
                                            scalar1=scl2[:, :], scalar2=shf2[:, :],
                                            op0=ALU.mult, op1=ALU.add)

            filler(fill["C"])
            c3open = c2keys[:-1]
            c3late = c2keys[-1]
            c3ps = {}
            for (s, ko, half) in c3open:
                ps = psum(f"c3_{s}{ko}{half}")
                c3ps[(s, ko, half)] = ps
                ps3 = ps[:, :].rearrange("p (r c) -> p r c", c=W)
                conv_part(2, h2_pad, s, ko, half, 0, ps3, first=True, last=False)

            scl3, shf3 = bn_consts(3, 1, 1)
            with nc.allow_low_precision(reason="f32r==f32 bit layout"):
                for s in range(BL):
                    nc.vector.tensor_scalar(out=_interior(h2_pad[s][1]),
                                            in0=_interior(h2_pad[s][1]),
                                            scalar1=scl3[:, :], scalar2=shf3[:, :],
                                            op0=ALU.mult, op1=ALU.add)

            y_sb = [[py.tile([128, N], dt_attn, tag="y", name=f"y{s}{k}")
                     for k in range(KT)] for s in range(BL)]
            filler(fill["D"])
            for (s, ko, half) in c2keys:
                if (s, ko, half) == c3late:
                    ps = psum(f"c3_{s}{ko}{half}")
                    ps3 = ps[:, :].rearrange("p (r c) -> p r c", c=W)
                    for ki in range(KT):
                        conv_part(2, h2_pad, s, ko, half, ki, ps3,
                                  first=(ki == 0), last=(ki == KT - 1))
                else:
                    ps = c3ps[(s, ko, half)]
                    ps3 = ps[:, :].rearrange("p (r c) -> p r c", c=W)
                    conv_part(2, h2_pad, s, ko, half, 1, ps3,
                              first=False, last=True)
                nc.scalar.activation(
                    y_sb[s][ko][:, half * 512:(half + 1) * 512],
                    ps[:, :], AF.Identity, bias=cb_sb[2][ko][:, :])

            # ================= attention (per sample) ======================
            filler(fill["E"])
            for s in range(BL):
                vt = []
                filler(fill["V"])
                for nt in range(8):
                    ps = psum(f"vps{s}{nt}")
                    pv = ps[:, 0:C]
                    for c2 in range(KT):
                        nc.tensor.matmul(pv, y_sb[s][c2][:, nt * 128:(nt + 1) * 128],
                                         wvt_sb[c2][:, :], start=(c2 == 0), stop=False)
                    nc.tensor.matmul(pv, ones_row[:, :], bv_sb[:, :],
                                     start=False, stop=True)
                    v = pat.tile([128, C], dt_attn, tag="vt", bufs=9, name=f"vt{s}{nt}")
                    nc.vector.tensor_copy(v[:, :], pv)
                    vt.append(v)

                q_sb = pat.tile([CQ, N], dt_attn, tag="q", bufs=2, name=f"q{s}")
                k_sb = pat.tile([CQ, N], dt_attn, tag="k", bufs=2, name=f"k{s}")
                for nh in range(2):
                    psq_ = psum(f"qps{s}{nh}")
                    for c2 in range(KT):
                        nc.tensor.matmul(psq_[0:CQ, :], wqt_sb[c2][:, :],
                                         y_sb[s][c2][:, nh * 512:(nh + 1) * 512],
                                         start=(c2 == 0), stop=(c2 == KT - 1))
                    nc.scalar.activation(q_sb[:, nh * 512:(nh + 1) * 512],
                                         psq_[0:CQ, :], AF.Identity, bias=bq_sb[:, :])
                    psk_ = psum(f"kps{s}{nh}")
                    for c2 in range(KT):
                        nc.tensor.matmul(psk_[0:CQ, :], wkt_sb[c2][:, :],
                                         y_sb[s][c2][:, nh * 512:(nh + 1) * 512],
                                         start=(c2 == 0), stop=(c2 == KT - 1))
                    nc.scalar.activation(k_sb[:, nh * 512:(nh + 1) * 512],
                                         psk_[0:CQ, :], AF.Identity, bias=bk_sb[:, :])

                res_t = pat.tile([128, KT * N], F32R, tag="res", bufs=2,
                                 name=f"res{s}")
                for nh in range(2):
                    filler(fill["S"])
                    ptiles = []
                    for mt in range(8):
                        ps = psum(f"sps{s}{nh}{mt}")
                        nc.tensor.matmul(ps[:, :], k_sb[:, mt * 128:(mt + 1) * 128],
                                         q_sb[:, nh * 512:(nh + 1) * 512],
                                         start=True, stop=True)
                        p = pat.tile([128, 512], dt_attn, tag="P", bufs=12,
                                     name=f"P{s}{nh}{mt}")
                        nc.scalar.activation(p[:, :], ps[:, :], AF.Exp)
                        ptiles.append(p)
                    # V @ P output matmuls first in PE order
                    pr_ps = []
                    for c2 in range(KT):
                        pr = psum(f"rps{s}{nh}{c2}")
                        pr_ps.append(pr)
                        for mt in range(8):
                            nc.tensor.matmul(pr[:, :],
                                             vt[mt][:, c2 * 128:(c2 + 1) * 128],
                                             ptiles[mt][:, :],
                                             start=(mt == 0), stop=(mt == 7))
                    # denominator: two DVE add-trees + 2 accumulating
                    # ones-matmuls (short tail)
                    pacc = [pat.tile([128, 512], dt_attn, tag="pacc", bufs=2,
                                     name=f"pacc{s}{nh}{h}") for h in range(2)]
                    for h in range(2):
                        nc.vector.tensor_tensor(pacc[h][:, :],
                                                ptiles[4 * h][:, :],
                                                ptiles[4 * h + 1][:, :], ALU.add)
                        nc.vector.tensor_tensor(pacc[h][:, :], pacc[h][:, :],
                                                ptiles[4 * h + 2][:, :], ALU.add)
                        nc.vector.tensor_tensor(pacc[h][:, :], pacc[h][:, :],
                                                ptiles[4 * h + 3][:, :], ALU.add)
                    pd = psum(f"dps{s}{nh}")
                    nc.tensor.matmul(pd[0:1, :], ones_col[:, :], pacc[0][:, :],
                                     start=True, stop=False)
                    nc.tensor.matmul(pd[0:1, :], ones_col[:, :], pacc[1][:, :],
                                     start=False, stop=True)
                    rcp = pat.tile([1, 512], dt_attn, tag="rcp", bufs=2,
                                   name=f"rcp{s}{nh}")
                    with nc.allow_low_precision(reason="f32r==f32 bit layout"):
                        nc.vector.reciprocal(rcp[:, :], pd[0:1, :])
                    pb = psum(f"bps{s}{nh}")
                    nc.tensor.matmul(pb[:, :], ones_row[:, :], rcp[:, :],
                                     start=True, stop=True)
                    rb = pat.tile([128, 512], F32, tag="rb", bufs=2, name=f"rb{s}{nh}")
                    nc.vector.tensor_copy(rb[:, :], pb[:, :])
                    # out = (V @ P) * rb + y  (gamma folded into wv/bv on host)
                    for c2 in range(KT):
                        rs = res_t[:, c2 * N + nh * 512:c2 * N + (nh + 1) * 512]
                        nc.vector.tensor_tensor(rs, pr_ps[c2][:, :], rb[:, :],
                                                ALU.mult)
                        nc.vector.tensor_tensor(rs, rs,
                                                y_sb[s][c2][:, nh * 512:(nh + 1) * 512],
                                                ALU.add)
                    nc.sync.dma_start(
                        out=out_d[s, :, :, nh * 512:(nh + 1) * 512]
                            .rearrange("k p n -> p k n"),
                        in_=res_t[:, :].rearrange("p (k n) -> p k n", k=KT)
                            [:, :, nh * 512:(nh + 1) * 512])

    if split:
        _split_packed_waits(nc)
    return nc


def _prep_inputs(inputs):
    import ml_dtypes
    bf16 = ml_dtypes.bfloat16 if USE_BF16 else np.float32
    f32 = np.float32
    x = np.asarray(inputs["x"], f32)
    t = np.asarray(inputs["t"], f32)

    def conv_w(w):
        w6 = np.asarray(w, f32).reshape(KT, 128, KT, 128, 3, 3)  # ko,o,ki,i,dy,dx
        arr = w6.transpose(3, 0, 4, 5, 2, 1)  # i,ko,dy,dx,ki,o
        return np.ascontiguousarray(arr.reshape(128, KT * KCOLS))

    cwa = np.stack([conv_w(inputs["w_c1"]),
                    conv_w(inputs["w_c2"])]).astype(bf16)
    cw2 = conv_w(inputs["w_tr"])
    w1t = np.ascontiguousarray(np.asarray(inputs["w_t1"], f32).T.reshape(KT, 128, T))
    w2t = np.ascontiguousarray(np.asarray(inputs["w_t2"], f32).T.reshape(KT, 128, C))
    consts = np.zeros((128, 22), f32)
    for ci, k2 in enumerate(("b_c1", "b_c2", "b_tr")):
        consts[:, ci * KT:(ci + 1) * KT] = np.asarray(inputs[k2], f32).reshape(KT, 128).T
    for i, (gk, bk2) in enumerate((("bn1_g", "bn1_b"), ("bn2_g", "bn2_b"))):
        consts[:, 6 + i * KT:6 + (i + 1) * KT] = np.asarray(inputs[gk], f32).reshape(KT, 128).T
        consts[:, 10 + i * KT:10 + (i + 1) * KT] = np.asarray(inputs[bk2], f32).reshape(KT, 128).T
    consts[:, 14:16] = np.asarray(inputs["b_t1"], f32).reshape(KT, 128).T
    consts[:, 16:18] = np.asarray(inputs["b_t2"], f32).reshape(KT, 128).T
    gam = float(np.asarray(inputs["gamma"], f32).reshape(()))
    wqt = np.ascontiguousarray(np.asarray(inputs["wq"], f32).T.reshape(KT, 128, CQ))
    wkt = np.ascontiguousarray(np.asarray(inputs["wk"], f32).T.reshape(KT, 128, CQ))
    # gamma folded into V projection (out = gamma*out_att + y)
    wvt = np.ascontiguousarray(
        (gam * np.asarray(inputs["wv"], f32)).T.reshape(KT, 128, C))
    bq = np.asarray(inputs["bq"], f32).reshape(CQ, 1)
    bk = np.asarray(inputs["bk"], f32).reshape(CQ, 1)
    bv = (gam * np.asarray(inputs["bv"], f32)).reshape(1, C)

    xp = np.zeros((B, KT, 128, HP, WP), bf16)
    xp[:, :, :, 1:1 + H, 1:1 + W] = x.reshape(B, KT, 128, H, W).astype(bf16)
    xp = xp.reshape(B, KT, 128, NPAD)
    ttr = np.ascontiguousarray(t.T.reshape(KT, 128, B))

    shared = dict(cwa=cwa, cw2=cw2, w1t=w1t, w2t=w2t,
                  wqt=wqt, wkt=wkt, wvt=wvt, bq=bq, bk=bk, bv=bv)
    per_core = []
    for c in range(NCORES):
        m = dict(shared)
        m["xp"] = np.ascontiguousarray(xp[c * BL:(c + 1) * BL])
        cc_consts = consts.copy()
        for k in range(KT):
            cc_consts[:, 18 + k * BL:18 + (k + 1) * BL] = \
                ttr[k, :, c * BL:(c + 1) * BL]
        m["consts"] = cc_consts
        per_core.append(m)
    return per_core


def _unshard(results):
    out = np.empty((B, C, H, W), np.float32)
    for c in range(NCORES):
        o = results[c]["out"].reshape(BL, KT, 128, H, W)
        for s in range(BL):
            out[c * BL + s] = o[s].reshape(C, H, W)
    return out


_cache = {}

DT_CONV = F32R
DT_ATTN = F32R


def kernel(**inputs) -> np.ndarray:
    key = ("nc", str(DT_CONV), str(DT_ATTN))
    if key not in _cache:
        _cache[key] = build(dt_conv=DT_CONV, dt_attn=DT_ATTN)
    nc = _cache[key]
    per_core = _prep_inputs(inputs)
    try:
        res = run_bass_kernel_spmd(nc, per_core, core_ids=list(range(NCORES)))
    except Exception:
        # transient NRT_EXEC_UNIT_UNRECOVERABLE errors recover on re-execute
        res = run_bass_kernel_spmd(nc, per_core, core_ids=list(range(NCORES)))
    return _unshard(res.results)
